# revision 17
# baseline (speedup 1.0000x reference)
"""MixHopVolatilityNet Trainium2 kernel (8 NeuronCores, SPMD).

Strategy (graph/data parallel, per sharding hint):
 - Nodes partitioned across 8 cores (1250 each) via a degree-balanced
   permutation; each core owns the destination side of every propagation
   for its nodes. Weights replicated.
 - Halo exchange: after each hop every core AllGathers its 1250-row slab
   into the next full [10000, F] feature table (on-chip ncfw collective).
 - Every hop runs as gather + segment-matmul per 128-dst-node block: the
   (deduplicated, per-block) source rows of the replicated table are
   batch-gathered into SBUF k-tiles with gpsimd.dma_gather (one Q7
   instruction per <=1024 rows, ~1us desc-gen each instead of the ~17us
   of per-128-row indirect DMAs), then PE accumulates
   wsegT[k_slot, dst]^T @ gathered[k_slot, :] over the k-tiles.
 - Layer 0 propagates h directly (propagate-then-project, 3x256-wide
   hops). Layers 1-2 project first (out_p = A^p (h @ W_p) + b_p),
   batching powers into [z1|z2|z3] so hops are 768/512/256 wide.
 - Per-hop staging of the AllGather input slab is batched into one SBUF
   tile and written with 2 DMAs (not 10) to cut HWDGE/queue overhead.
 - All matmul operands fp16 (PSUM accumulates fp32); layernorm (two-pass,
   bn_stats/bn_aggr) in fp32; erf-gelu via the ACT Gelu LUT.
"""

import heapq
import sys

import numpy as np

sys.path.insert(0, "/opt/trn_rl_repo")

# ---- problem constants (hardcoded per contract) ----
N = 10000
E = 160000
F_IN = 84
H = 256
P4 = 4
L = 3
PH = P4 * H  # 1024
NC = 8
NB = N // NC          # 1250 nodes per core
BLK = 128
NBLK = (NB + BLK - 1) // BLK   # 10 blocks; the last one holds 98 nodes
LAST = NB - (NBLK - 1) * BLK   # 98
EPS = 1e-5
GMAX = 8              # k-tiles per dma_gather (1024 rows = SWDGE ring cap)


def _nb_of(b):
    return min(BLK, NB - b * BLK)


# ----------------------------------------------------------------------------
# Host-side preprocessing
# ----------------------------------------------------------------------------

def _balance_nodes(wt):
    """Greedy LPT assignment of nodes to the 80 (core, block) bins so the
    per-block gather work is balanced. Returns perm: orig node -> new id."""
    nbins = NC * NBLK
    cap = np.full(nbins, BLK, np.int64)
    cap[NBLK - 1:: NBLK] = LAST
    order = np.argsort(-wt, kind="stable")
    heap = [(0, b) for b in range(nbins)]
    heapq.heapify(heap)
    fill = np.zeros(nbins, np.int64)
    perm = np.empty(N, np.int64)
    base = np.arange(nbins) // NBLK * NB + np.arange(nbins) % NBLK * BLK
    for node in order:
        while True:
            load, b = heapq.heappop(heap)
            if fill[b] < cap[b]:
                break
        perm[node] = base[b] + fill[b]
        fill[b] += 1
        if fill[b] < cap[b]:
            heapq.heappush(heap, (load + int(wt[node]), b))
    return perm


def _graph_prep(edge_index):
    """Build per-core gather index planes and dense segment-weight blocks,
    with dst-side node balancing and per-block source deduplication."""
    src = edge_index[0].astype(np.int64)
    dst = edge_index[1].astype(np.int64)
    deg = np.bincount(dst, minlength=N).astype(np.float64) + 1.0
    dinv = deg ** -0.5
    loop = np.arange(N, dtype=np.int64)
    esrc = np.concatenate([src, loop])
    edst = np.concatenate([dst, loop])
    ew = (dinv[esrc] * dinv[edst]).astype(np.float32)

    perm = _balance_nodes(deg)  # deg ~ per-dst gather row count
    psrc = perm[esrc]
    pdst = perm[edst]

    core = pdst // NB
    loc = pdst - core * NB
    blk = loc // BLK
    m = loc - blk * BLK
    gid = core * NBLK + blk
    order = np.argsort(gid, kind="stable")
    psrc, ew, m, gid = psrc[order], ew[order], m[order], gid[order]
    starts = np.searchsorted(gid, np.arange(NC * NBLK))
    ends = np.concatenate([starts[1:], [len(gid)]])

    # per-block dedup of gather sources
    uniq_lists = []
    kk = np.empty(len(gid), np.int64)
    counts = np.empty(NC * NBLK, np.int64)
    for g in range(NC * NBLK):
        s, e = starts[g], ends[g]
        u, inv = np.unique(psrc[s:e], return_inverse=True)
        uniq_lists.append(u)
        kk[s:e] = inv
        counts[g] = len(u)

    k_pad = int(np.ceil(max(counts.max(), 128) / 128.0) * 128)
    T = k_pad // 128

    wsegT = np.zeros((NC, 128, NBLK, T, BLK), np.float32)
    core_g = gid // NBLK
    blk_g = gid % NBLK
    np.add.at(wsegT, (core_g, kk % 128, blk_g, kk // 128, m), ew)
    wsegT = wsegT.astype(np.float16)

    # dma_gather index planes: slot i = t*128+p lands at out[i%128, i//128];
    # the ucode reads idxs[i] from plane[i%16, i//16], replicated across all
    # eight 16-partition groups (each Q7 core reads its own partitions).
    idx16 = np.zeros((NC, 128, NBLK, 8 * T), np.int16)
    for g in range(NC * NBLK):
        u = uniq_lists[g]
        arr = np.zeros(k_pad, np.int16)
        arr[: len(u)] = u.astype(np.int16)
        plane = arr.reshape(8 * T, 16).T          # [16, 8T]
        idx16[g // NBLK, :, g % NBLK, :] = np.tile(plane, (8, 1))
    return wsegT, idx16, k_pad, perm


def _w_moving(w):
    """[K, Nout] -> moving layout [128, Kt, Nout] fp16 (partition = K % 128)."""
    K, Nout = w.shape
    Kt = (K + 127) // 128
    out = np.zeros((128, Kt, Nout), np.float16)
    for t in range(Kt):
        rows = w[t * 128: min((t + 1) * 128, K)]
        out[: rows.shape[0], t] = rows.astype(np.float16)
    return out


def _w_stationary(w):
    """[K, M] -> stationary tiles [128, Kt, Mt, 128] fp16."""
    K, M = w.shape
    Kt = (K + 127) // 128
    Mt = (M + 127) // 128
    out = np.zeros((128, Kt, Mt, 128), np.float16)
    for t in range(Kt):
        for u in range(Mt):
            blk = w[t * 128: min((t + 1) * 128, K),
                    u * 128: min((u + 1) * 128, M)].astype(np.float16)
            out[: blk.shape[0], t, u, : blk.shape[1]] = blk
    return out


# ----------------------------------------------------------------------------
# Bass program
# ----------------------------------------------------------------------------


def _build_nc(k_pad, nontriv, use_collectives=True):
    import concourse.bacc as bacc
    import concourse.bass as bass
    import concourse.mybir as mybir
    import concourse.tile as tile
    from concourse.alu_op_type import AluOpType
    from contextlib import ExitStack

    f16 = mybir.dt.float16
    f32 = mybir.dt.float32
    f8 = mybir.dt.float8e4
    u8 = mybir.dt.uint8
    i16 = mybir.dt.int16
    AF = mybir.ActivationFunctionType
    T = k_pad // 128
    CH = [(q * GMAX, min((q + 1) * GMAX, T))
          for q in range((T + GMAX - 1) // GMAX)]
    RG = [list(range(NC))]

    nc = bacc.Bacc("TRN2", target_bir_lowering=False, debug=False,
                   num_devices=NC)

    # ---- I/O ----
    xT_d = nc.dram_tensor("xT", [F_IN, NB], f16, kind="ExternalInput")
    idx_d = nc.dram_tensor("idx16", [128, NBLK, 8 * T], i16,
                           kind="ExternalInput")
    wseg_d = nc.dram_tensor("wsegT", [128, NBLK, T, BLK], f16,
                            kind="ExternalInput")
    w_in_d = nc.dram_tensor("w_in_m", [128, 1, H], f16, kind="ExternalInput")
    w0_d = nc.dram_tensor("w0_m", [P4, 128, 2, H], f16, kind="ExternalInput")
    w12_d = nc.dram_tensor("w12_m", [2, P4, 128, 8, H], f16,
                           kind="ExternalInput")
    w1_d = nc.dram_tensor("w1_st", [128, 8, 2, 128], f16, kind="ExternalInput")
    w2_d = nc.dram_tensor("w2_st", [128, 2, 1, 128], f16, kind="ExternalInput")
    w3_d = nc.dram_tensor("w3_st", [128, 1], f16, kind="ExternalInput")
    ident_d = nc.dram_tensor("ident", [128, 128], f16, kind="ExternalInput")
    eps_d = nc.dram_tensor("eps_bc", [128, 1], f32, kind="ExternalInput")
    if nontriv["b_in"]:
        b_in_d = nc.dram_tensor("b_in_bc", [128, H], f32, kind="ExternalInput")
    if nontriv["bcat"]:
        bcat_d = nc.dram_tensor("bcat_bc", [L, 128, PH], f32,
                                kind="ExternalInput")
    if nontriv["ln"]:
        lng_d = nc.dram_tensor("lng_bc", [L, 128, PH], f32,
                               kind="ExternalInput")
        lnb_d = nc.dram_tensor("lnb_bc", [L, 128, PH], f32,
                               kind="ExternalInput")
    if nontriv["b1"]:
        b1_d = nc.dram_tensor("b1_c", [128, 2], f32, kind="ExternalInput")
    if nontriv["b2"]:
        b2_d = nc.dram_tensor("b2_c", [128, 1], f32, kind="ExternalInput")
    y_d = nc.dram_tensor("y_out", [NB], f32, kind="ExternalOutput")

    # ---- internal DRAM: AG inputs (local) and gather tables (shared) ----
    # fp8 plan: the z1 stage rows are PACKED bytes [256 fp16 | 512 fp8]:
    # the z1 columns (which feed power 1 directly and are most
    # error-sensitive) stay fp16, while [z2|z3] travel fp8e4m3 (their fp8
    # noise is carried through 2-3 further propagation hops); one u8
    # gather per chunk serves both via bitcast views. The [Az2|Az3] stage
    # is fp8 too. 256-wide tables stay fp16 (sub-512B descriptors pay a
    # 2x latency multiplier, so fp8 would not shrink their time).
    ag_in = {}
    table = {}
    tdt = {}
    for name, width, dt in [
            ("l0h0", H, f16), ("l0h1", H, f16), ("l0h2", H, f16),
            ("l1z1", 4 * H, u8), ("l1z2", 2 * H, f8), ("l1z3", H, f16),
            ("l2z1", 4 * H, u8), ("l2z2", 2 * H, f8), ("l2z3", H, f16)]:
        tdt[name] = dt
        ag_in[name] = nc.dram_tensor(f"agin_{name}", [NB, width], dt)
        table[name] = nc.dram_tensor(f"tab_{name}", [N, width], dt,
                                     addr_space="Shared")

    with tile.TileContext(nc) as tc, ExitStack() as ctx:
        const = ctx.enter_context(tc.tile_pool(name="const", bufs=1))
        work = ctx.enter_context(tc.tile_pool(name="work", bufs=2))
        big = ctx.enter_context(tc.tile_pool(name="big", bufs=1))
        gpool = ctx.enter_context(tc.tile_pool(name="gpool", bufs=3))
        one = ctx.enter_context(tc.tile_pool(name="one", bufs=1))
        psum = ctx.enter_context(tc.tile_pool(name="psum", bufs=6,
                                              space="PSUM"))
        pstr = ctx.enter_context(tc.tile_pool(name="pstr", bufs=2,
                                              space="PSUM"))

        # ---- persistent SBUF constants (h0 operands first) ----
        xT_sb = const.tile([F_IN, NB], f16, tag="xT")
        nc.sync.dma_start(out=xT_sb[:], in_=xT_d[:])
        w_in_sb = const.tile([128, 1, H], f16, tag="w_in")
        nc.sync.dma_start(out=w_in_sb[:], in_=w_in_d[:])
        ident_sb = const.tile([128, 128], f16, tag="ident")
        nc.sync.dma_start(out=ident_sb[:], in_=ident_d[:])
        eps_sb = const.tile([128, 1], f32, tag="eps")
        nc.sync.dma_start(out=eps_sb[:], in_=eps_d[:])
        zero_sb = const.tile([128, 1], f32, tag="zero")
        nc.vector.memset(zero_sb[:], 0.0)
        wseg_sb = const.tile([128, NBLK, T, BLK], f16, tag="wseg")
        nc.sync.dma_start(out=wseg_sb[:], in_=wseg_d[:])
        idx_sb = const.tile([128, NBLK, 8 * T], i16, tag="idx")
        nc.sync.dma_start(out=idx_sb[:], in_=idx_d[:])
        w0_sb = const.tile([128, P4, 2, H], f16, tag="w0")
        for p in range(P4):
            nc.sync.dma_start(out=w0_sb[:, p, :, :], in_=w0_d[p])
        w1_sb = const.tile([128, 8, 2, 128], f16, tag="w1")
        nc.sync.dma_start(out=w1_sb[:], in_=w1_d[:])
        w2_sb = const.tile([128, 2, 1, 128], f16, tag="w2")
        nc.sync.dma_start(out=w2_sb[:], in_=w2_d[:])
        w3_sb = const.tile([128, 1], f16, tag="w3")
        nc.sync.dma_start(out=w3_sb[:], in_=w3_d[:])
        if nontriv["b_in"]:
            b_in_sb = const.tile([128, H], f32, tag="b_in")
            nc.sync.dma_start(out=b_in_sb[:], in_=b_in_d[:])
        if nontriv["bcat"]:
            bcat_sb = const.tile([128, L, PH], f32, tag="bcat")
            for i in range(L):
                nc.sync.dma_start(out=bcat_sb[:, i, :], in_=bcat_d[i])
        if nontriv["ln"]:
            lng_sb = const.tile([128, L, PH], f32, tag="lng")
            lnb_sb = const.tile([128, L, PH], f32, tag="lnb")
            for i in range(L):
                nc.sync.dma_start(out=lng_sb[:, i, :], in_=lng_d[i])
                nc.sync.dma_start(out=lnb_sb[:, i, :], in_=lnb_d[i])
        if nontriv["b1"]:
            b1_sb = const.tile([128, 2], f32, tag="b1")
            nc.sync.dma_start(out=b1_sb[:], in_=b1_d[:])
        if nontriv["b2"]:
            b2_sb = const.tile([128, 1], f32, tag="b2")
            nc.sync.dma_start(out=b2_sb[:], in_=b2_d[:])

        # persistent activations. During layer 0, hT[:, 2p:2p+2, :] holds the
        # feature-major transpose of A^p h (the hops' projection operands);
        # after each layernorm it holds the feature-major layer output.
        hT = big.tile([128, 8, NB], f16, tag="hT")
        hcat = big.tile([128, NBLK, PH], f16, tag="hcat")
        # node-major staging slabs for the next AllGather input (batched
        # write): fp16 for 256-wide slabs, fp8 for the wide (768/512) ones
        stg = big.tile([128, NBLK, 2 * H], f16, tag="stg")
        stg8 = big.tile([128, NBLK, 2 * H], f8, tag="stg8")

        def zb(nb):
            return zero_sb[:nb, 0:1]

        def allgather(name):
            """Halo exchange ag_in[name] -> table[name]. With collectives off
            (cost-model timing builds, which can't model ncfw), stand in a
            local DMA with the same per-core HBM write volume."""
            if use_collectives:
                nc.gpsimd.collective_compute(
                    "AllGather", AluOpType.bypass, replica_groups=RG,
                    ins=[ag_in[name][:]], outs=[table[name][:]],
                )
            else:
                for c in range(2):
                    nc.sync.dma_start(
                        out=table[name][c * NB:(c + 1) * NB, :],
                        in_=ag_in[name][:])

        def stage_flush(name, width, slab):
            """Write the staged [NB, width] slab to ag_in[name] in 2 DMAs."""
            full = (NBLK - 1) * BLK  # 1152
            nc.sync.dma_start(
                out=ag_in[name][:full, :].rearrange("(a p) f -> p a f", p=128),
                in_=slab[:, : NBLK - 1, :width])
            nc.sync.dma_start(
                out=ag_in[name][full:, :],
                in_=slab[:LAST, NBLK - 1, :width])

        def transposes_to(dst_ap3, src_ap2, nb, nkt):
            """dst[128, nkt, nb] (feature-major) = per-k-tile transposes of
            src[nb, nkt*128] via PE, staged through one PSUM bank and copied
            out with a single (strided) DVE copy."""
            pst = pstr.tile([128, nkt, 128], f16, tag="tr")
            for kt in range(nkt):
                nc.tensor.transpose(pst[:, kt, :nb],
                                    src_ap2[:nb, kt * 128:(kt + 1) * 128],
                                    ident_sb[:nb, :nb])
            nc.vector.tensor_copy(dst_ap3, pst[:, :, :nb])

        def seg_psums(tabs, b):
            """Propagation block b: dma_gather the block's (deduplicated)
            source rows of each table in `tabs` into k-tiles, then
            segment-matmul on PE. Each entry of `tabs` is (tab, views)
            where views = [(byte_lo, dtype, ncols)] reinterprets byte
            ranges of the gathered rows (for the packed fp16|fp8 z1
            table); plain tables pass one full-width view. Psum pieces
            are aligned to view boundaries; returns [(c0, cw, psum_tile)]
            over the concatenated column space."""
            outs = []
            srcs = []          # (tab_idx, view) per psum piece
            base = 0
            for ti, (tab, views) in enumerate(tabs):
                for (blo, vdt, ncols) in views:
                    c0 = 0
                    while c0 < ncols:
                        cw = min(512, ncols - c0)
                        ps = psum.tile([128, 512], f32, tag="mm",
                                       name="ps_seg")
                        outs.append((base + c0, cw, ps))
                        srcs.append((ti, blo, vdt, c0))
                        c0 += cw
                    base += ncols
            for (t0, t1) in CH:
                gts = []
                for (tab, views) in tabs:
                    width = tab.shape[1]
                    gt = gpool.tile([128, t1 - t0, width], tab.dtype,
                                    tag="gath")
                    nc.gpsimd.dma_gather(
                        out_ap=gt[:],
                        in_ap=tab[:],
                        idxs_ap=idx_sb[:, b, t0 * 8: t1 * 8],
                        num_idxs=(t1 - t0) * 128,
                        num_idxs_reg=(t1 - t0) * 128,
                        elem_size=width,
                    )
                    gts.append((gt, tab.dtype))
                for (_, cw, ps), (ti, blo, vdt, tc0) in zip(outs, srcs):
                    gt, gdt = gts[ti]
                    gsz = mybir.dt.size(gdt)
                    lo = (blo + tc0 * mybir.dt.size(vdt)) // gsz
                    ncnt = cw * mybir.dt.size(vdt) // gsz
                    for t in range(t0, t1):
                        rhs = gt[:, t - t0, lo: lo + ncnt]
                        if vdt != gdt:
                            rhs = rhs.bitcast(vdt)
                        nc.tensor.matmul(
                            ps[:, :cw],
                            wseg_sb[:, b, t, :],
                            rhs,
                            start=(t == 0),
                            stop=(t == T - 1),
                        )
            return outs

        def ln_pass1(layer, b):
            """hcat[:, b] -> (+bias) -> bn stats -> mv (mu, var, std, rstd,
            -mu*rstd). Issued inside the last hop/projection of the layer so
            the Sqrts run while ACT is otherwise idle (one table load)."""
            hc = hcat[:, b, :]
            if nontriv["bcat"]:
                nc.vector.tensor_tensor(hc, hc, bcat_sb[:, layer, :],
                                        AluOpType.add)
            st = work.tile([128, 12], f32, tag="bnst", name="st")
            nc.vector.bn_stats(st[:, 0:6], hcat[:, b, 0:512])
            nc.vector.bn_stats(st[:, 6:12], hcat[:, b, 512:1024])
            mv = work.tile([128, 6], f32, tag=f"bnmv{b}", name="mv")
            nc.vector.bn_aggr(mv[:, 0:2], st[:])
            nc.scalar.activation(mv[:, 2:3], mv[:, 1:2], AF.Sqrt,
                                 bias=eps_sb[:, 0:1])
            nc.vector.reciprocal(mv[:, 3:4], mv[:, 2:3])
            # mv[:,4] = -mu * rstd, the fused-activation bias
            nc.vector.tensor_scalar(mv[:, 4:5], mv[:, 0:1], mv[:, 3:4],
                                    -1.0, AluOpType.mult, AluOpType.mult)
            return mv

        def ln_pass2(layer, mvs, next_blk=None):
            """normalize+gelu (one fused ACT op per block) -> hT transposes,
            interleaving `next_blk(b)` (next-layer projections / MLP chunks)
            so PE never drains between blocks."""
            for b in range(NBLK):
                nb = _nb_of(b)
                mv = mvs[b]
                gl = work.tile([128, PH], f16, tag="gel")
                if nontriv["ln"]:
                    xn = one.tile([128, PH], f32, tag="xn")
                    nc.vector.tensor_scalar(
                        xn[:], hcat[:, b, :], mv[:, 0:1], mv[:, 3:4],
                        AluOpType.subtract, AluOpType.mult,
                    )
                    nc.vector.tensor_tensor(xn[:], xn[:],
                                            lng_sb[:, layer, :],
                                            AluOpType.mult)
                    nc.vector.tensor_tensor(xn[:], xn[:],
                                            lnb_sb[:, layer, :],
                                            AluOpType.add)
                    nc.scalar.activation(gl[:], xn[:], AF.Gelu, bias=zb(128))
                else:
                    # gelu((x - mu) * rstd) == gelu(x * rstd + (-mu * rstd))
                    nc.scalar.activation(gl[:], hcat[:, b, :], AF.Gelu,
                                         bias=mv[:, 4:5], scale=mv[:, 3:4])
                transposes_to(hT[:, :, b * BLK: b * BLK + nb], gl, nb, 8)
                if next_blk is not None:
                    next_blk(b)

        # ================= stage 0: h0 = gelu(x @ w_in + b_in) =============
        for b in range(NBLK):
            nb = _nb_of(b)
            ps = psum.tile([128, 512], f32, tag="mm")
            nc.tensor.matmul(ps[:nb, :H],
                             xT_sb[:, b * BLK: b * BLK + nb],
                             w_in_sb[:F_IN, 0, :], start=True, stop=True)
            if nontriv["b_in"]:
                tmp = work.tile([128, 512], f32, tag="btmp")
                nc.vector.tensor_tensor(tmp[:nb, :H], ps[:nb, :H],
                                        b_in_sb[:nb, :], AluOpType.add)
                nc.scalar.activation(stg[:nb, b, :H], tmp[:nb, :H], AF.Gelu,
                                     bias=zb(nb))
            else:
                nc.scalar.activation(stg[:nb, b, :H], ps[:nb, :H], AF.Gelu,
                                     bias=zb(nb))
            transposes_to(hT[:, 0:2, b * BLK: b * BLK + nb],
                          stg[:, b, :H], nb, 2)
        stage_flush("l0h0", H, stg)
        allgather("l0h0")

        # ================= layer 0: propagate-then-project =================
        mvs = [None] * NBLK

        def l0_project(p):
            """hcat[:, b, p*H:(p+1)*H] = h_p @ mh_w0[p] from hT[:, 2p:2p+2].
            The p=3 pass completes hcat, so it chains each block's LN
            stats (ln_pass1) right behind its projection."""
            for b in range(NBLK):
                nb = _nb_of(b)
                ps = psum.tile([128, 512], f32, tag="mm")
                for kt in range(2):
                    nc.tensor.matmul(ps[:nb, :H],
                                     hT[:, 2 * p + kt, b * BLK: b * BLK + nb],
                                     w0_sb[:, p, kt, :],
                                     start=(kt == 0), stop=(kt == 1))
                nc.vector.tensor_copy(hcat[:nb, b, p * H:(p + 1) * H],
                                      ps[:nb, :H])
                if p == 3:
                    mvs[b] = ln_pass1(0, b)

        l0_project(0)
        hops = [("l0h0", "l0h1"), ("l0h1", "l0h2"), ("l0h2", None)]
        for p, (tin, tout) in enumerate(hops, start=1):
            for b in range(NBLK):
                nb = _nb_of(b)
                (_, _, ps), = seg_psums([(table[tin], [(0, f16, H)])], b)
                if tout is not None:
                    nc.vector.tensor_copy(stg[:, b, :H], ps[:, :H])
                    transposes_to(hT[:, 2 * p: 2 * p + 2, b * BLK: b * BLK + nb],
                                  stg[:, b, :H], nb, 2)
                else:
                    sg = work.tile([128, H], f16, tag="sg")
                    nc.vector.tensor_copy(sg[:], ps[:, :H])
                    transposes_to(hT[:, 2 * p: 2 * p + 2, b * BLK: b * BLK + nb],
                                  sg[:, :H], nb, 2)
            if tout is not None:
                stage_flush(tout, H, stg)
                allgather(tout)
            l0_project(p)

        # ================= layers 1-2: project-first ======================
        def make_proj(layer, w12_sb):
            def proj(b):
                """MixHop projections for `layer`: p=0 -> hcat, p=1 -> fp16
                z1 staging, p=2,3 -> fp8 staging (AG input)."""
                nb = _nb_of(b)
                for p in range(P4):
                    ps = psum.tile([128, 512], f32, tag="mm")
                    for kt in range(8):
                        nc.tensor.matmul(ps[:nb, :H],
                                         hT[:, kt, b * BLK: b * BLK + nb],
                                         w12_sb[:, p, kt, :],
                                         start=(kt == 0), stop=(kt == 7))
                    if p == 0:
                        nc.vector.tensor_copy(hcat[:nb, b, 0:H], ps[:nb, :H])
                    elif p == 1:
                        nc.vector.tensor_copy(stg[:nb, b, :H], ps[:nb, :H])
                    else:
                        nc.vector.tensor_copy(
                            stg8[:nb, b, (p - 2) * H: (p - 1) * H],
                            ps[:nb, :H])
            return proj

        def z1_flush_ag(zn):
            # packed z1 flush: fp16 z1 bytes [0:512), fp8 [z2|z3] [512:1024)
            agz = ag_in[zn]
            full = (NBLK - 1) * BLK
            bulk = agz[:full, :].rearrange("(a p) f -> p a f", p=128)
            nc.sync.dma_start(out=bulk[:, :, 0: 2 * H].bitcast(f16),
                              in_=stg[:, : NBLK - 1, :H])
            nc.sync.dma_start(out=bulk[:, :, 2 * H: 4 * H].bitcast(f8),
                              in_=stg8[:, : NBLK - 1, : 2 * H])
            nc.sync.dma_start(out=agz[full:, 0: 2 * H].bitcast(f16),
                              in_=stg[:LAST, NBLK - 1, :H])
            nc.sync.dma_start(out=agz[full:, 2 * H: 4 * H].bitcast(f8),
                              in_=stg8[:LAST, NBLK - 1, : 2 * H])
            allgather(zn)

        def run_hops(layer):
            """The three propagation hops of a MixHop layer; the last hop
            chains per-block LN stats."""
            zname = [f"l{layer}z1", f"l{layer}z2", f"l{layer}z3"]
            hop_tabs = [[(table[zname[0]],
                          [(0, f16, H), (2 * H, f8, 2 * H)])],
                        [(table[zname[1]], [(0, f8, 2 * H)])],
                        [(table[zname[2]], [(0, f16, H)])]]
            for hop in range(3):
                width = (3 - hop) * H
                tout = zname[hop + 1] if hop < 2 else None
                for b in range(NBLK):
                    nb = _nb_of(b)
                    pieces = seg_psums(hop_tabs[hop], b)
                    # first H columns are this hop's power output
                    nc.vector.tensor_copy(
                        hcat[:nb, b, (hop + 1) * H:(hop + 2) * H],
                        pieces[0][2][:nb, :H])
                    if tout is not None:
                        oslab = stg8 if tdt[tout] == f8 else stg
                        for (c0, cw, ps) in pieces:
                            if c0 + cw <= H:
                                continue
                            lo = max(H, c0)
                            nc.vector.tensor_copy(
                                oslab[:, b, lo - H: c0 + cw - H],
                                ps[:, lo - c0: cw])
                    else:
                        mvs[b] = ln_pass1(layer, b)
                if tout is not None:
                    stage_flush(tout, width - H,
                                stg8 if tdt[tout] == f8 else stg)
                    allgather(tout)

        # layer 1 projections interleave with layer 0's LN pass 2
        w12_sb1 = const.tile([128, P4, 8, H], f16, tag="w12a")
        for p in range(P4):
            nc.sync.dma_start(out=w12_sb1[:, p, :, :], in_=w12_d[0, p])
        ln_pass2(0, mvs, make_proj(1, w12_sb1))
        z1_flush_ag("l1z1")
        run_hops(1)

        w12_sb2 = const.tile([128, P4, 8, H], f16, tag="w12b")
        for p in range(P4):
            nc.sync.dma_start(out=w12_sb2[:, p, :, :], in_=w12_d[1, p])
        ln_pass2(1, mvs, make_proj(2, w12_sb2))
        z1_flush_ag("l2z1")
        run_hops(2)

        # ============ final MLP, interleaved with layer 2's LN ============
        m1T = big.tile([128, 2, NB], f16, tag="stg", name="m1T")
        m2T = big.tile([128, NB], f16, tag="hcat", name="m2T")
        chunks = [(c, min(512, NB - c)) for c in range(0, NB, 512)]

        def mlp_chunk(ci):
            c0, cw = chunks[ci]
            for mt in range(2):
                ps = psum.tile([128, 512], f32, tag="mm")
                for kt in range(8):
                    nc.tensor.matmul(ps[:, :cw], w1_sb[:, kt, mt, :],
                                     hT[:, kt, c0:c0 + cw],
                                     start=(kt == 0), stop=(kt == 7))
                bias = b1_sb[:, mt:mt + 1] if nontriv["b1"] else zb(128)
                nc.scalar.activation(m1T[:, mt, c0:c0 + cw], ps[:, :cw],
                                     AF.Gelu, bias=bias)
            ps = psum.tile([128, 512], f32, tag="mm")
            for kt in range(2):
                nc.tensor.matmul(ps[:, :cw], w2_sb[:, kt, 0, :],
                                 m1T[:, kt, c0:c0 + cw],
                                 start=(kt == 0), stop=(kt == 1))
            bias = b2_sb[:, 0:1] if nontriv["b2"] else zb(128)
            nc.scalar.activation(m2T[:, c0:c0 + cw], ps[:, :cw],
                                 AF.Gelu, bias=bias)

        def mlp_blk(b):
            # chunk ci spans hT columns [ci*512, ci*512+512) -> ready once
            # blocks 0..b cover them
            if b == 3:
                mlp_chunk(0)
            elif b == 7:
                mlp_chunk(1)
            elif b == 9:
                mlp_chunk(2)

        ln_pass2(2, mvs, mlp_blk)
        ysb = big.tile([1, NB], f32, tag="ysb", name="ysb")
        for (c0, cw) in chunks:
            ps = psum.tile([128, 512], f32, tag="mm")
            nc.tensor.matmul(ps[:1, :cw], w3_sb[:, :1], m2T[:, c0:c0 + cw],
                             start=True, stop=True)
            nc.vector.tensor_copy(ysb[:1, c0:c0 + cw], ps[:1, :cw])
        nc.sync.dma_start(out=y_d[:], in_=ysb[:1, :])

    nc.compile()
    return nc


# ----------------------------------------------------------------------------
# Public entry point
# ----------------------------------------------------------------------------

_CACHE = {}


def _prep_inputs(inputs):
    x = np.asarray(inputs["x"], np.float32)
    edge_index = np.asarray(inputs["edge_index"])
    wsegT, idx16, k_pad, perm = _graph_prep(edge_index)

    b3 = np.asarray(inputs["b3"], np.float32)
    nontriv = {
        "b_in": bool(np.any(inputs["b_in"])),
        "bcat": bool(np.any(inputs["mh_b0"]) or np.any(inputs["mh_b12"])),
        "ln": not (np.allclose(np.asarray(inputs["ln_g"]), 1.0)
                   and not np.any(inputs["ln_b"])),
        "b1": bool(np.any(inputs["b1"])),
        "b2": bool(np.any(inputs["b2"])),
    }

    shared = {
        "w_in_m": _w_moving(np.asarray(inputs["w_in"], np.float32)),
        "w0_m": np.stack([_w_moving(np.asarray(inputs["mh_w0"][p], np.float32))
                          for p in range(P4)]),
        "w12_m": np.stack([
            np.stack([_w_moving(np.asarray(inputs["mh_w12"][l, p], np.float32))
                      for p in range(P4)])
            for l in range(2)]),
        "w1_st": _w_stationary(np.asarray(inputs["w1"], np.float32)),
        "w2_st": _w_stationary(np.asarray(inputs["w2"], np.float32)),
        "w3_st": np.asarray(inputs["w3"], np.float32).astype(np.float16),
        "ident": np.eye(128, dtype=np.float16),
        "eps_bc": np.full((128, 1), EPS, np.float32),
    }
    if nontriv["b_in"]:
        shared["b_in_bc"] = np.tile(np.asarray(inputs["b_in"], np.float32),
                                    (128, 1))
    if nontriv["bcat"]:
        bcat = np.zeros((L, PH), np.float32)
        bcat[0] = np.asarray(inputs["mh_b0"], np.float32).reshape(-1)
        bcat[1] = np.asarray(inputs["mh_b12"], np.float32)[0].reshape(-1)
        bcat[2] = np.asarray(inputs["mh_b12"], np.float32)[1].reshape(-1)
        shared["bcat_bc"] = np.ascontiguousarray(
            np.broadcast_to(bcat[:, None, :], (L, 128, PH)))
    if nontriv["ln"]:
        shared["lng_bc"] = np.ascontiguousarray(np.broadcast_to(
            np.asarray(inputs["ln_g"], np.float32)[:, None, :], (L, 128, PH)))
        shared["lnb_bc"] = np.ascontiguousarray(np.broadcast_to(
            np.asarray(inputs["ln_b"], np.float32)[:, None, :], (L, 128, PH)))
    if nontriv["b1"]:
        shared["b1_c"] = np.ascontiguousarray(
            np.asarray(inputs["b1"], np.float32).reshape(2, 128).T)
    if nontriv["b2"]:
        shared["b2_c"] = np.asarray(inputs["b2"], np.float32).reshape(128, 1)

    xp = x[np.argsort(perm)]  # xp[newid] = x[orig]
    in_maps = []
    for c in range(NC):
        m = dict(shared)
        m["xT"] = np.ascontiguousarray(
            xp[c * NB:(c + 1) * NB].T.astype(np.float16))
        m["idx16"] = np.ascontiguousarray(idx16[c])
        m["wsegT"] = np.ascontiguousarray(wsegT[c])
        in_maps.append(m)
    return in_maps, k_pad, nontriv, b3, perm


def _run(inputs, trace=False, **kwargs):
    from concourse.bass_utils import run_bass_kernel_spmd

    in_maps, k_pad, nontriv, b3, perm = _prep_inputs(inputs)
    key = (k_pad, tuple(sorted(nontriv.items())))
    if key not in _CACHE:
        _CACHE[key] = _build_nc(k_pad, nontriv)
    nc = _CACHE[key]
    res = run_bass_kernel_spmd(nc, in_maps, list(range(NC)), trace=trace,
                               **kwargs)
    ycat = np.concatenate([res.results[c]["y_out"] for c in range(NC)])
    y = ycat[perm].astype(np.float32) + b3.reshape(-1)[0]
    return y, res


def kernel(**inputs) -> np.ndarray:
    y, _ = _run(inputs, trace=False)
    return y


# revision 18
# speedup vs baseline: 1.0080x; 1.0080x over previous
"""MixHopVolatilityNet Trainium2 kernel (8 NeuronCores, SPMD).

Strategy (graph/data parallel, per sharding hint):
 - Nodes partitioned across 8 cores (1250 each) via a degree-balanced
   permutation; each core owns the destination side of every propagation
   for its nodes. Weights replicated.
 - Halo exchange: after each hop every core AllGathers its 1250-row slab
   into the next full [10000, F] feature table (on-chip ncfw collective).
 - Every hop runs as gather + segment-matmul per 128-dst-node block: the
   (deduplicated, per-block) source rows of the replicated table are
   batch-gathered into SBUF k-tiles with gpsimd.dma_gather (one Q7
   instruction per <=1024 rows, ~1us desc-gen each instead of the ~17us
   of per-128-row indirect DMAs), then PE accumulates
   wsegT[k_slot, dst]^T @ gathered[k_slot, :] over the k-tiles.
 - Layer 0 propagates h directly (propagate-then-project, 3x256-wide
   hops). Layers 1-2 project first (out_p = A^p (h @ W_p) + b_p),
   batching powers into [z1|z2|z3] so hops are 768/512/256 wide.
 - Per-hop staging of the AllGather input slab is batched into one SBUF
   tile and written with 2 DMAs (not 10) to cut HWDGE/queue overhead.
 - All matmul operands fp16 (PSUM accumulates fp32); layernorm (two-pass,
   bn_stats/bn_aggr) in fp32; erf-gelu via the ACT Gelu LUT.
"""

import heapq
import sys

import numpy as np

sys.path.insert(0, "/opt/trn_rl_repo")

# ---- problem constants (hardcoded per contract) ----
N = 10000
E = 160000
F_IN = 84
H = 256
P4 = 4
L = 3
PH = P4 * H  # 1024
NC = 8
NB = N // NC          # 1250 nodes per core
BLK = 128
NBLK = (NB + BLK - 1) // BLK   # 10 blocks; the last one holds 98 nodes
LAST = NB - (NBLK - 1) * BLK   # 98
EPS = 1e-5
GMAX = 8              # k-tiles per dma_gather (1024 rows = SWDGE ring cap)


def _nb_of(b):
    return min(BLK, NB - b * BLK)


# ----------------------------------------------------------------------------
# Host-side preprocessing
# ----------------------------------------------------------------------------

def _balance_nodes(wt):
    """Greedy LPT assignment of nodes to the 80 (core, block) bins so the
    per-block gather work is balanced. Returns perm: orig node -> new id."""
    nbins = NC * NBLK
    cap = np.full(nbins, BLK, np.int64)
    cap[NBLK - 1:: NBLK] = LAST
    order = np.argsort(-wt, kind="stable")
    heap = [(0, b) for b in range(nbins)]
    heapq.heapify(heap)
    fill = np.zeros(nbins, np.int64)
    perm = np.empty(N, np.int64)
    base = np.arange(nbins) // NBLK * NB + np.arange(nbins) % NBLK * BLK
    for node in order:
        while True:
            load, b = heapq.heappop(heap)
            if fill[b] < cap[b]:
                break
        perm[node] = base[b] + fill[b]
        fill[b] += 1
        if fill[b] < cap[b]:
            heapq.heappush(heap, (load + int(wt[node]), b))
    return perm


def _graph_prep(edge_index):
    """Build per-core gather index planes and dense segment-weight blocks,
    with dst-side node balancing and per-block source deduplication."""
    src = edge_index[0].astype(np.int64)
    dst = edge_index[1].astype(np.int64)
    deg = np.bincount(dst, minlength=N).astype(np.float64) + 1.0
    dinv = deg ** -0.5
    loop = np.arange(N, dtype=np.int64)
    esrc = np.concatenate([src, loop])
    edst = np.concatenate([dst, loop])
    ew = (dinv[esrc] * dinv[edst]).astype(np.float32)

    perm = _balance_nodes(deg)  # deg ~ per-dst gather row count
    psrc = perm[esrc]
    pdst = perm[edst]

    core = pdst // NB
    loc = pdst - core * NB
    blk = loc // BLK
    m = loc - blk * BLK
    gid = core * NBLK + blk
    order = np.argsort(gid, kind="stable")
    psrc, ew, m, gid = psrc[order], ew[order], m[order], gid[order]
    starts = np.searchsorted(gid, np.arange(NC * NBLK))
    ends = np.concatenate([starts[1:], [len(gid)]])

    # per-block dedup of gather sources
    uniq_lists = []
    kk = np.empty(len(gid), np.int64)
    counts = np.empty(NC * NBLK, np.int64)
    for g in range(NC * NBLK):
        s, e = starts[g], ends[g]
        u, inv = np.unique(psrc[s:e], return_inverse=True)
        uniq_lists.append(u)
        kk[s:e] = inv
        counts[g] = len(u)

    k_pad = int(np.ceil(max(counts.max(), 128) / 128.0) * 128)
    T = k_pad // 128

    wsegT = np.zeros((NC, 128, NBLK, T, BLK), np.float32)
    core_g = gid // NBLK
    blk_g = gid % NBLK
    np.add.at(wsegT, (core_g, kk % 128, blk_g, kk // 128, m), ew)
    wsegT = wsegT.astype(np.float16)

    # dma_gather index planes: slot i = t*128+p lands at out[i%128, i//128];
    # the ucode reads idxs[i] from plane[i%16, i//16], replicated across all
    # eight 16-partition groups (each Q7 core reads its own partitions).
    idx16 = np.zeros((NC, 128, NBLK, 8 * T), np.int16)
    for g in range(NC * NBLK):
        u = uniq_lists[g]
        arr = np.zeros(k_pad, np.int16)
        arr[: len(u)] = u.astype(np.int16)
        plane = arr.reshape(8 * T, 16).T          # [16, 8T]
        idx16[g // NBLK, :, g % NBLK, :] = np.tile(plane, (8, 1))
    return wsegT, idx16, k_pad, perm


def _w_moving(w):
    """[K, Nout] -> moving layout [128, Kt, Nout] fp16 (partition = K % 128)."""
    K, Nout = w.shape
    Kt = (K + 127) // 128
    out = np.zeros((128, Kt, Nout), np.float16)
    for t in range(Kt):
        rows = w[t * 128: min((t + 1) * 128, K)]
        out[: rows.shape[0], t] = rows.astype(np.float16)
    return out


def _w_stationary(w):
    """[K, M] -> stationary tiles [128, Kt, Mt, 128] fp16."""
    K, M = w.shape
    Kt = (K + 127) // 128
    Mt = (M + 127) // 128
    out = np.zeros((128, Kt, Mt, 128), np.float16)
    for t in range(Kt):
        for u in range(Mt):
            blk = w[t * 128: min((t + 1) * 128, K),
                    u * 128: min((u + 1) * 128, M)].astype(np.float16)
            out[: blk.shape[0], t, u, : blk.shape[1]] = blk
    return out


# ----------------------------------------------------------------------------
# Bass program
# ----------------------------------------------------------------------------


def _build_nc(k_pad, nontriv, use_collectives=True):
    import concourse.bacc as bacc
    import concourse.bass as bass
    import concourse.mybir as mybir
    import concourse.tile as tile
    from concourse.alu_op_type import AluOpType
    from contextlib import ExitStack

    f16 = mybir.dt.float16
    f32 = mybir.dt.float32
    f8 = mybir.dt.float8e4
    u8 = mybir.dt.uint8
    i16 = mybir.dt.int16
    AF = mybir.ActivationFunctionType
    T = k_pad // 128
    CH = [(q * GMAX, min((q + 1) * GMAX, T))
          for q in range((T + GMAX - 1) // GMAX)]
    RG = [list(range(NC))]

    nc = bacc.Bacc("TRN2", target_bir_lowering=False, debug=False,
                   num_devices=NC)

    # ---- I/O ----
    xT_d = nc.dram_tensor("xT", [F_IN, NB], f16, kind="ExternalInput")
    idx_d = nc.dram_tensor("idx16", [128, NBLK, 8 * T], i16,
                           kind="ExternalInput")
    wseg_d = nc.dram_tensor("wsegT", [128, NBLK, T, BLK], f16,
                            kind="ExternalInput")
    w_in_d = nc.dram_tensor("w_in_m", [128, 1, H], f16, kind="ExternalInput")
    w0_d = nc.dram_tensor("w0_m", [P4, 128, 2, H], f16, kind="ExternalInput")
    w12_d = nc.dram_tensor("w12_m", [2, P4, 128, 8, H], f16,
                           kind="ExternalInput")
    w1_d = nc.dram_tensor("w1_st", [128, 8, 2, 128], f16, kind="ExternalInput")
    w2_d = nc.dram_tensor("w2_st", [128, 2, 1, 128], f16, kind="ExternalInput")
    w3_d = nc.dram_tensor("w3_st", [128, 1], f16, kind="ExternalInput")
    ident_d = nc.dram_tensor("ident", [128, 128], f16, kind="ExternalInput")
    eps_d = nc.dram_tensor("eps_bc", [128, 1], f32, kind="ExternalInput")
    if nontriv["b_in"]:
        b_in_d = nc.dram_tensor("b_in_bc", [128, H], f32, kind="ExternalInput")
    if nontriv["bcat"]:
        bcat_d = nc.dram_tensor("bcat_bc", [L, 128, PH], f32,
                                kind="ExternalInput")
    if nontriv["ln"]:
        lng_d = nc.dram_tensor("lng_bc", [L, 128, PH], f32,
                               kind="ExternalInput")
        lnb_d = nc.dram_tensor("lnb_bc", [L, 128, PH], f32,
                               kind="ExternalInput")
    if nontriv["b1"]:
        b1_d = nc.dram_tensor("b1_c", [128, 2], f32, kind="ExternalInput")
    if nontriv["b2"]:
        b2_d = nc.dram_tensor("b2_c", [128, 1], f32, kind="ExternalInput")
    y_d = nc.dram_tensor("y_out", [NB], f32, kind="ExternalOutput")

    # ---- internal DRAM: AG inputs (local) and gather tables (shared) ----
    # fp8 plan: the z1 stage rows are PACKED bytes [256 fp16 | 512 fp8]:
    # the z1 columns (which feed power 1 directly and are most
    # error-sensitive) stay fp16, while [z2|z3] travel fp8e4m3 (their fp8
    # noise is carried through 2-3 further propagation hops); one u8
    # gather per chunk serves both via bitcast views. The [Az2|Az3] stage
    # is fp8 too. 256-wide tables stay fp16 (sub-512B descriptors pay a
    # 2x latency multiplier, so fp8 would not shrink their time).
    ag_in = {}
    table = {}
    tdt = {}
    for name, width, dt in [
            ("l0h0", H, f16), ("l0h1", H, f16), ("l0h2", H, f16),
            ("l1z1", 4 * H, u8), ("l1z2", 2 * H, f8), ("l1z3", H, f16),
            ("l2z1", 4 * H, u8), ("l2z2", 2 * H, f8), ("l2z3", H, f16)]:
        tdt[name] = dt
        ag_in[name] = nc.dram_tensor(f"agin_{name}", [NB, width], dt)
        table[name] = nc.dram_tensor(f"tab_{name}", [N, width], dt,
                                     addr_space="Shared")

    with tile.TileContext(nc) as tc, ExitStack() as ctx:
        const = ctx.enter_context(tc.tile_pool(name="const", bufs=1))
        work = ctx.enter_context(tc.tile_pool(name="work", bufs=2))
        big = ctx.enter_context(tc.tile_pool(name="big", bufs=1))
        gpool = ctx.enter_context(tc.tile_pool(name="gpool", bufs=3))
        one = ctx.enter_context(tc.tile_pool(name="one", bufs=1))
        psum = ctx.enter_context(tc.tile_pool(name="psum", bufs=6,
                                              space="PSUM"))
        pstr = ctx.enter_context(tc.tile_pool(name="pstr", bufs=2,
                                              space="PSUM"))

        # ---- persistent SBUF constants (h0 operands first) ----
        xT_sb = const.tile([F_IN, NB], f16, tag="xT")
        nc.sync.dma_start(out=xT_sb[:], in_=xT_d[:])
        w_in_sb = const.tile([128, 1, H], f16, tag="w_in")
        nc.sync.dma_start(out=w_in_sb[:], in_=w_in_d[:])
        ident_sb = const.tile([128, 128], f16, tag="ident")
        nc.sync.dma_start(out=ident_sb[:], in_=ident_d[:])
        eps_sb = const.tile([128, 1], f32, tag="eps")
        nc.sync.dma_start(out=eps_sb[:], in_=eps_d[:])
        zero_sb = const.tile([128, 1], f32, tag="zero")
        nc.vector.memset(zero_sb[:], 0.0)
        wseg_sb = const.tile([128, NBLK, T, BLK], f16, tag="wseg")
        nc.sync.dma_start(out=wseg_sb[:], in_=wseg_d[:])
        idx_sb = const.tile([128, NBLK, 8 * T], i16, tag="idx")
        nc.sync.dma_start(out=idx_sb[:], in_=idx_d[:])
        w0_sb = const.tile([128, P4, 2, H], f16, tag="w0")
        for p in range(P4):
            nc.sync.dma_start(out=w0_sb[:, p, :, :], in_=w0_d[p])
        w1_sb = const.tile([128, 8, 2, 128], f16, tag="w1")
        nc.sync.dma_start(out=w1_sb[:], in_=w1_d[:])
        w2_sb = const.tile([128, 2, 1, 128], f16, tag="w2")
        nc.sync.dma_start(out=w2_sb[:], in_=w2_d[:])
        w3_sb = const.tile([128, 1], f16, tag="w3")
        nc.sync.dma_start(out=w3_sb[:], in_=w3_d[:])
        if nontriv["b_in"]:
            b_in_sb = const.tile([128, H], f32, tag="b_in")
            nc.sync.dma_start(out=b_in_sb[:], in_=b_in_d[:])
        if nontriv["bcat"]:
            bcat_sb = const.tile([128, L, PH], f32, tag="bcat")
            for i in range(L):
                nc.sync.dma_start(out=bcat_sb[:, i, :], in_=bcat_d[i])
        if nontriv["ln"]:
            lng_sb = const.tile([128, L, PH], f32, tag="lng")
            lnb_sb = const.tile([128, L, PH], f32, tag="lnb")
            for i in range(L):
                nc.sync.dma_start(out=lng_sb[:, i, :], in_=lng_d[i])
                nc.sync.dma_start(out=lnb_sb[:, i, :], in_=lnb_d[i])
        if nontriv["b1"]:
            b1_sb = const.tile([128, 2], f32, tag="b1")
            nc.sync.dma_start(out=b1_sb[:], in_=b1_d[:])
        if nontriv["b2"]:
            b2_sb = const.tile([128, 1], f32, tag="b2")
            nc.sync.dma_start(out=b2_sb[:], in_=b2_d[:])

        # persistent activations. During layer 0, hT[:, 2p:2p+2, :] holds the
        # feature-major transpose of A^p h (the hops' projection operands);
        # after each layernorm it holds the feature-major layer output.
        hT = big.tile([128, 8, NB], f16, tag="hT")
        hcat = big.tile([128, NBLK, PH], f16, tag="hcat")
        # node-major staging slabs for the next AllGather input (batched
        # write): fp16 for 256-wide slabs, fp8 for the wide (768/512) ones
        stg = big.tile([128, NBLK, 2 * H], f16, tag="stg")
        stg8 = big.tile([128, NBLK, 2 * H], f8, tag="stg8")

        def zb(nb):
            return zero_sb[:nb, 0:1]

        def allgather(name):
            """Halo exchange ag_in[name] -> table[name]. With collectives off
            (cost-model timing builds, which can't model ncfw), stand in a
            local DMA with the same per-core HBM write volume."""
            if use_collectives:
                nc.gpsimd.collective_compute(
                    "AllGather", AluOpType.bypass, replica_groups=RG,
                    ins=[ag_in[name][:]], outs=[table[name][:]],
                )
            else:
                for c in range(2):
                    nc.sync.dma_start(
                        out=table[name][c * NB:(c + 1) * NB, :],
                        in_=ag_in[name][:])

        def stage_flush(name, width, slab):
            """Write the staged [NB, width] slab to ag_in[name] in 2 DMAs."""
            full = (NBLK - 1) * BLK  # 1152
            nc.sync.dma_start(
                out=ag_in[name][:full, :].rearrange("(a p) f -> p a f", p=128),
                in_=slab[:, : NBLK - 1, :width])
            nc.sync.dma_start(
                out=ag_in[name][full:, :],
                in_=slab[:LAST, NBLK - 1, :width])

        def transposes_to(dst_ap3, src_ap2, nb, nkt):
            """dst[128, nkt, nb] (feature-major) = per-k-tile transposes of
            src[nb, nkt*128] via PE, staged through one PSUM bank and copied
            out with a single (strided) DVE copy."""
            pst = pstr.tile([128, nkt, 128], f16, tag="tr")
            for kt in range(nkt):
                nc.tensor.transpose(pst[:, kt, :nb],
                                    src_ap2[:nb, kt * 128:(kt + 1) * 128],
                                    ident_sb[:nb, :nb])
            nc.vector.tensor_copy(dst_ap3, pst[:, :, :nb])

        def seg_psums(tabs, b):
            """Propagation block b: dma_gather the block's (deduplicated)
            source rows of each table in `tabs` into k-tiles, then
            segment-matmul on PE. Each entry of `tabs` is (tab, views)
            where views = [(byte_lo, dtype, ncols)] reinterprets byte
            ranges of the gathered rows (for the packed fp16|fp8 z1
            table); plain tables pass one full-width view. Psum pieces
            are aligned to view boundaries; returns [(c0, cw, psum_tile)]
            over the concatenated column space."""
            outs = []
            srcs = []          # (tab_idx, view) per psum piece
            base = 0
            for ti, (tab, views) in enumerate(tabs):
                for (blo, vdt, ncols) in views:
                    c0 = 0
                    while c0 < ncols:
                        cw = min(512, ncols - c0)
                        ps = psum.tile([128, 512], f32, tag="mm",
                                       name="ps_seg")
                        outs.append((base + c0, cw, ps))
                        srcs.append((ti, blo, vdt, c0))
                        c0 += cw
                    base += ncols
            for (t0, t1) in CH:
                gts = []
                for (tab, views) in tabs:
                    width = tab.shape[1]
                    gt = gpool.tile([128, t1 - t0, width], tab.dtype,
                                    tag="gath")
                    nc.gpsimd.dma_gather(
                        out_ap=gt[:],
                        in_ap=tab[:],
                        idxs_ap=idx_sb[:, b, t0 * 8: t1 * 8],
                        num_idxs=(t1 - t0) * 128,
                        num_idxs_reg=(t1 - t0) * 128,
                        elem_size=width,
                    )
                    gts.append((gt, tab.dtype))
                for (_, cw, ps), (ti, blo, vdt, tc0) in zip(outs, srcs):
                    gt, gdt = gts[ti]
                    gsz = mybir.dt.size(gdt)
                    lo = (blo + tc0 * mybir.dt.size(vdt)) // gsz
                    ncnt = cw * mybir.dt.size(vdt) // gsz
                    for t in range(t0, t1):
                        rhs = gt[:, t - t0, lo: lo + ncnt]
                        if vdt != gdt:
                            rhs = rhs.bitcast(vdt)
                        nc.tensor.matmul(
                            ps[:, :cw],
                            wseg_sb[:, b, t, :],
                            rhs,
                            start=(t == 0),
                            stop=(t == T - 1),
                        )
            return outs

        def ln_pass1(layer, b):
            """hcat[:, b] -> (+bias) -> bn stats -> mv (mu, var, std, rstd,
            -mu*rstd). Issued inside the last hop/projection of the layer so
            the Sqrts run while ACT is otherwise idle (one table load)."""
            hc = hcat[:, b, :]
            if nontriv["bcat"]:
                nc.vector.tensor_tensor(hc, hc, bcat_sb[:, layer, :],
                                        AluOpType.add)
            st = work.tile([128, 12], f32, tag="bnst", name="st")
            nc.vector.bn_stats(st[:, 0:6], hcat[:, b, 0:512])
            nc.vector.bn_stats(st[:, 6:12], hcat[:, b, 512:1024])
            mv = work.tile([128, 6], f32, tag=f"bnmv{b}", name="mv")
            nc.vector.bn_aggr(mv[:, 0:2], st[:])
            nc.scalar.activation(mv[:, 2:3], mv[:, 1:2], AF.Sqrt,
                                 bias=eps_sb[:, 0:1])
            nc.vector.reciprocal(mv[:, 3:4], mv[:, 2:3])
            # mv[:,4] = -mu * rstd, the fused-activation bias
            nc.vector.tensor_scalar(mv[:, 4:5], mv[:, 0:1], mv[:, 3:4],
                                    -1.0, AluOpType.mult, AluOpType.mult)
            return mv

        def ln_pass2(layer, mvs, next_blk=None):
            """normalize+gelu (one fused ACT op per block) -> hT transposes,
            interleaving `next_blk(b)` (next-layer projections / MLP chunks)
            so PE never drains between blocks."""
            for b in range(NBLK):
                nb = _nb_of(b)
                mv = mvs[b]
                gl = work.tile([128, PH], f16, tag="gel")
                if nontriv["ln"]:
                    xn = one.tile([128, PH], f32, tag="xn")
                    nc.vector.tensor_scalar(
                        xn[:], hcat[:, b, :], mv[:, 0:1], mv[:, 3:4],
                        AluOpType.subtract, AluOpType.mult,
                    )
                    nc.vector.tensor_tensor(xn[:], xn[:],
                                            lng_sb[:, layer, :],
                                            AluOpType.mult)
                    nc.vector.tensor_tensor(xn[:], xn[:],
                                            lnb_sb[:, layer, :],
                                            AluOpType.add)
                    nc.scalar.activation(gl[:], xn[:], AF.Gelu, bias=zb(128))
                else:
                    # gelu((x - mu) * rstd) == gelu(x * rstd + (-mu * rstd))
                    nc.scalar.activation(gl[:], hcat[:, b, :], AF.Gelu,
                                         bias=mv[:, 4:5], scale=mv[:, 3:4])
                transposes_to(hT[:, :, b * BLK: b * BLK + nb], gl, nb, 8)
                if next_blk is not None:
                    next_blk(b)

        # ================= stage 0: h0 = gelu(x @ w_in + b_in) =============
        for b in range(NBLK):
            nb = _nb_of(b)
            ps = psum.tile([128, 512], f32, tag="mm")
            nc.tensor.matmul(ps[:nb, :H],
                             xT_sb[:, b * BLK: b * BLK + nb],
                             w_in_sb[:F_IN, 0, :], start=True, stop=True)
            if nontriv["b_in"]:
                tmp = work.tile([128, 512], f32, tag="btmp")
                nc.vector.tensor_tensor(tmp[:nb, :H], ps[:nb, :H],
                                        b_in_sb[:nb, :], AluOpType.add)
                nc.scalar.activation(stg[:nb, b, :H], tmp[:nb, :H], AF.Gelu,
                                     bias=zb(nb))
            else:
                nc.scalar.activation(stg[:nb, b, :H], ps[:nb, :H], AF.Gelu,
                                     bias=zb(nb))
            transposes_to(hT[:, 0:2, b * BLK: b * BLK + nb],
                          stg[:, b, :H], nb, 2)
        stage_flush("l0h0", H, stg)
        allgather("l0h0")

        # ================= layer 0: propagate-then-project =================
        mvs = [None] * NBLK

        def l0_project(p):
            """hcat[:, b, p*H:(p+1)*H] = h_p @ mh_w0[p] from hT[:, 2p:2p+2].
            The p=3 pass completes hcat, so it chains each block's LN
            stats (ln_pass1) right behind its projection."""
            for b in range(NBLK):
                nb = _nb_of(b)
                ps = psum.tile([128, 512], f32, tag="mm")
                for kt in range(2):
                    nc.tensor.matmul(ps[:nb, :H],
                                     hT[:, 2 * p + kt, b * BLK: b * BLK + nb],
                                     w0_sb[:, p, kt, :],
                                     start=(kt == 0), stop=(kt == 1))
                nc.vector.tensor_copy(hcat[:nb, b, p * H:(p + 1) * H],
                                      ps[:nb, :H])

        l0_project(0)
        hops = [("l0h0", "l0h1"), ("l0h1", "l0h2"), ("l0h2", None)]
        for p, (tin, tout) in enumerate(hops, start=1):
            for b in range(NBLK):
                nb = _nb_of(b)
                (_, _, ps), = seg_psums([(table[tin], [(0, f16, H)])], b)
                if tout is not None:
                    nc.vector.tensor_copy(stg[:, b, :H], ps[:, :H])
                    transposes_to(hT[:, 2 * p: 2 * p + 2, b * BLK: b * BLK + nb],
                                  stg[:, b, :H], nb, 2)
                else:
                    sg = work.tile([128, H], f16, tag="sg")
                    nc.vector.tensor_copy(sg[:], ps[:, :H])
                    transposes_to(hT[:, 2 * p: 2 * p + 2, b * BLK: b * BLK + nb],
                                  sg[:, :H], nb, 2)
            if tout is not None:
                stage_flush(tout, H, stg)
                allgather(tout)
            l0_project(p)

        # ================= layers 1-2: project-first ======================
        def make_proj(layer, w12_sb):
            def proj(b):
                """MixHop projections for `layer`: p=0 -> hcat, p=1 -> fp16
                z1 staging, p=2,3 -> fp8 staging (AG input)."""
                nb = _nb_of(b)
                for p in range(P4):
                    ps = psum.tile([128, 512], f32, tag="mm")
                    for kt in range(8):
                        nc.tensor.matmul(ps[:nb, :H],
                                         hT[:, kt, b * BLK: b * BLK + nb],
                                         w12_sb[:, p, kt, :],
                                         start=(kt == 0), stop=(kt == 7))
                    if p == 0:
                        nc.vector.tensor_copy(hcat[:nb, b, 0:H], ps[:nb, :H])
                    elif p == 1:
                        nc.vector.tensor_copy(stg[:nb, b, :H], ps[:nb, :H])
                    else:
                        nc.vector.tensor_copy(
                            stg8[:nb, b, (p - 2) * H: (p - 1) * H],
                            ps[:nb, :H])
            return proj

        def z1_flush_ag(zn):
            # packed z1 flush: fp16 z1 bytes [0:512), fp8 [z2|z3] [512:1024)
            agz = ag_in[zn]
            full = (NBLK - 1) * BLK
            bulk = agz[:full, :].rearrange("(a p) f -> p a f", p=128)
            nc.sync.dma_start(out=bulk[:, :, 0: 2 * H].bitcast(f16),
                              in_=stg[:, : NBLK - 1, :H])
            nc.sync.dma_start(out=bulk[:, :, 2 * H: 4 * H].bitcast(f8),
                              in_=stg8[:, : NBLK - 1, : 2 * H])
            nc.sync.dma_start(out=agz[full:, 0: 2 * H].bitcast(f16),
                              in_=stg[:LAST, NBLK - 1, :H])
            nc.sync.dma_start(out=agz[full:, 2 * H: 4 * H].bitcast(f8),
                              in_=stg8[:LAST, NBLK - 1, : 2 * H])
            allgather(zn)

        def run_hops(layer):
            """The three propagation hops of a MixHop layer; the last hop
            chains per-block LN stats."""
            zname = [f"l{layer}z1", f"l{layer}z2", f"l{layer}z3"]
            hop_tabs = [[(table[zname[0]],
                          [(0, f16, H), (2 * H, f8, 2 * H)])],
                        [(table[zname[1]], [(0, f8, 2 * H)])],
                        [(table[zname[2]], [(0, f16, H)])]]
            for hop in range(3):
                width = (3 - hop) * H
                tout = zname[hop + 1] if hop < 2 else None
                for b in range(NBLK):
                    nb = _nb_of(b)
                    pieces = seg_psums(hop_tabs[hop], b)
                    # first H columns are this hop's power output
                    nc.vector.tensor_copy(
                        hcat[:nb, b, (hop + 1) * H:(hop + 2) * H],
                        pieces[0][2][:nb, :H])
                    if tout is not None:
                        oslab = stg8 if tdt[tout] == f8 else stg
                        for (c0, cw, ps) in pieces:
                            if c0 + cw <= H:
                                continue
                            lo = max(H, c0)
                            nc.vector.tensor_copy(
                                oslab[:, b, lo - H: c0 + cw - H],
                                ps[:, lo - c0: cw])
                if tout is not None:
                    stage_flush(tout, width - H,
                                stg8 if tdt[tout] == f8 else stg)
                    allgather(tout)

        # layer 1 projections interleave with layer 0's LN pass 2
        w12_sb1 = const.tile([128, P4, 8, H], f16, tag="w12a")
        for p in range(P4):
            nc.sync.dma_start(out=w12_sb1[:, p, :, :], in_=w12_d[0, p])
        for b in range(NBLK):
            mvs[b] = ln_pass1(0, b)
        ln_pass2(0, mvs, make_proj(1, w12_sb1))
        z1_flush_ag("l1z1")
        run_hops(1)

        w12_sb2 = const.tile([128, P4, 8, H], f16, tag="w12b")
        for p in range(P4):
            nc.sync.dma_start(out=w12_sb2[:, p, :, :], in_=w12_d[1, p])
        for b in range(NBLK):
            mvs[b] = ln_pass1(1, b)
        ln_pass2(1, mvs, make_proj(2, w12_sb2))
        z1_flush_ag("l2z1")
        run_hops(2)

        # ============ final MLP, interleaved with layer 2's LN ============
        m1T = big.tile([128, 2, NB], f16, tag="stg", name="m1T")
        m2T = big.tile([128, NB], f16, tag="hcat", name="m2T")
        chunks = [(c, min(512, NB - c)) for c in range(0, NB, 512)]

        def mlp_chunk(ci):
            c0, cw = chunks[ci]
            for mt in range(2):
                ps = psum.tile([128, 512], f32, tag="mm")
                for kt in range(8):
                    nc.tensor.matmul(ps[:, :cw], w1_sb[:, kt, mt, :],
                                     hT[:, kt, c0:c0 + cw],
                                     start=(kt == 0), stop=(kt == 7))
                bias = b1_sb[:, mt:mt + 1] if nontriv["b1"] else zb(128)
                nc.scalar.activation(m1T[:, mt, c0:c0 + cw], ps[:, :cw],
                                     AF.Gelu, bias=bias)
            ps = psum.tile([128, 512], f32, tag="mm")
            for kt in range(2):
                nc.tensor.matmul(ps[:, :cw], w2_sb[:, kt, 0, :],
                                 m1T[:, kt, c0:c0 + cw],
                                 start=(kt == 0), stop=(kt == 1))
            bias = b2_sb[:, 0:1] if nontriv["b2"] else zb(128)
            nc.scalar.activation(m2T[:, c0:c0 + cw], ps[:, :cw],
                                 AF.Gelu, bias=bias)

        def mlp_blk(b):
            # chunk ci spans hT columns [ci*512, ci*512+512) -> ready once
            # blocks 0..b cover them
            if b == 3:
                mlp_chunk(0)
            elif b == 7:
                mlp_chunk(1)
            elif b == 9:
                mlp_chunk(2)

        for b in range(NBLK):
            mvs[b] = ln_pass1(2, b)
        ln_pass2(2, mvs, mlp_blk)
        ysb = big.tile([1, NB], f32, tag="ysb", name="ysb")
        for (c0, cw) in chunks:
            ps = psum.tile([128, 512], f32, tag="mm")
            nc.tensor.matmul(ps[:1, :cw], w3_sb[:, :1], m2T[:, c0:c0 + cw],
                             start=True, stop=True)
            nc.vector.tensor_copy(ysb[:1, c0:c0 + cw], ps[:1, :cw])
        nc.sync.dma_start(out=y_d[:], in_=ysb[:1, :])

    nc.compile()
    return nc


# ----------------------------------------------------------------------------
# Public entry point
# ----------------------------------------------------------------------------

_CACHE = {}


def _prep_inputs(inputs):
    x = np.asarray(inputs["x"], np.float32)
    edge_index = np.asarray(inputs["edge_index"])
    wsegT, idx16, k_pad, perm = _graph_prep(edge_index)

    b3 = np.asarray(inputs["b3"], np.float32)
    nontriv = {
        "b_in": bool(np.any(inputs["b_in"])),
        "bcat": bool(np.any(inputs["mh_b0"]) or np.any(inputs["mh_b12"])),
        "ln": not (np.allclose(np.asarray(inputs["ln_g"]), 1.0)
                   and not np.any(inputs["ln_b"])),
        "b1": bool(np.any(inputs["b1"])),
        "b2": bool(np.any(inputs["b2"])),
    }

    shared = {
        "w_in_m": _w_moving(np.asarray(inputs["w_in"], np.float32)),
        "w0_m": np.stack([_w_moving(np.asarray(inputs["mh_w0"][p], np.float32))
                          for p in range(P4)]),
        "w12_m": np.stack([
            np.stack([_w_moving(np.asarray(inputs["mh_w12"][l, p], np.float32))
                      for p in range(P4)])
            for l in range(2)]),
        "w1_st": _w_stationary(np.asarray(inputs["w1"], np.float32)),
        "w2_st": _w_stationary(np.asarray(inputs["w2"], np.float32)),
        "w3_st": np.asarray(inputs["w3"], np.float32).astype(np.float16),
        "ident": np.eye(128, dtype=np.float16),
        "eps_bc": np.full((128, 1), EPS, np.float32),
    }
    if nontriv["b_in"]:
        shared["b_in_bc"] = np.tile(np.asarray(inputs["b_in"], np.float32),
                                    (128, 1))
    if nontriv["bcat"]:
        bcat = np.zeros((L, PH), np.float32)
        bcat[0] = np.asarray(inputs["mh_b0"], np.float32).reshape(-1)
        bcat[1] = np.asarray(inputs["mh_b12"], np.float32)[0].reshape(-1)
        bcat[2] = np.asarray(inputs["mh_b12"], np.float32)[1].reshape(-1)
        shared["bcat_bc"] = np.ascontiguousarray(
            np.broadcast_to(bcat[:, None, :], (L, 128, PH)))
    if nontriv["ln"]:
        shared["lng_bc"] = np.ascontiguousarray(np.broadcast_to(
            np.asarray(inputs["ln_g"], np.float32)[:, None, :], (L, 128, PH)))
        shared["lnb_bc"] = np.ascontiguousarray(np.broadcast_to(
            np.asarray(inputs["ln_b"], np.float32)[:, None, :], (L, 128, PH)))
    if nontriv["b1"]:
        shared["b1_c"] = np.ascontiguousarray(
            np.asarray(inputs["b1"], np.float32).reshape(2, 128).T)
    if nontriv["b2"]:
        shared["b2_c"] = np.asarray(inputs["b2"], np.float32).reshape(128, 1)

    xp = x[np.argsort(perm)]  # xp[newid] = x[orig]
    in_maps = []
    for c in range(NC):
        m = dict(shared)
        m["xT"] = np.ascontiguousarray(
            xp[c * NB:(c + 1) * NB].T.astype(np.float16))
        m["idx16"] = np.ascontiguousarray(idx16[c])
        m["wsegT"] = np.ascontiguousarray(wsegT[c])
        in_maps.append(m)
    return in_maps, k_pad, nontriv, b3, perm


def _run(inputs, trace=False, **kwargs):
    from concourse.bass_utils import run_bass_kernel_spmd

    in_maps, k_pad, nontriv, b3, perm = _prep_inputs(inputs)
    key = (k_pad, tuple(sorted(nontriv.items())))
    if key not in _CACHE:
        _CACHE[key] = _build_nc(k_pad, nontriv)
    nc = _CACHE[key]
    res = run_bass_kernel_spmd(nc, in_maps, list(range(NC)), trace=trace,
                               **kwargs)
    ycat = np.concatenate([res.results[c]["y_out"] for c in range(NC)])
    y = ycat[perm].astype(np.float32) + b3.reshape(-1)[0]
    return y, res


def kernel(**inputs) -> np.ndarray:
    y, _ = _run(inputs, trace=False)
    return y


# revision 19
# speedup vs baseline: 1.0128x; 1.0048x over previous
"""MixHopVolatilityNet Trainium2 kernel (8 NeuronCores, SPMD).

Strategy (graph/data parallel, per sharding hint):
 - Nodes partitioned across 8 cores (1250 each) via a degree-balanced
   permutation; each core owns the destination side of every propagation
   for its nodes. Weights replicated.
 - Halo exchange: after each hop every core AllGathers its 1250-row slab
   into the next full [10000, F] feature table (on-chip ncfw collective).
 - Every hop runs as gather + segment-matmul per 128-dst-node block: the
   (deduplicated, per-block) source rows of the replicated table are
   batch-gathered into SBUF k-tiles with gpsimd.dma_gather (one Q7
   instruction per <=1024 rows, ~1us desc-gen each instead of the ~17us
   of per-128-row indirect DMAs), then PE accumulates
   wsegT[k_slot, dst]^T @ gathered[k_slot, :] over the k-tiles.
 - Layer 0 propagates h directly (propagate-then-project, 3x256-wide
   hops). Layers 1-2 project first (out_p = A^p (h @ W_p) + b_p),
   batching powers into [z1|z2|z3] so hops are 768/512/256 wide.
 - Per-hop staging of the AllGather input slab is batched into one SBUF
   tile and written with 2 DMAs (not 10) to cut HWDGE/queue overhead.
 - All matmul operands fp16 (PSUM accumulates fp32); layernorm (two-pass,
   bn_stats/bn_aggr) in fp32; erf-gelu via the ACT Gelu LUT.
"""

import heapq
import sys

import numpy as np

sys.path.insert(0, "/opt/trn_rl_repo")

# ---- problem constants (hardcoded per contract) ----
N = 10000
E = 160000
F_IN = 84
H = 256
P4 = 4
L = 3
PH = P4 * H  # 1024
NC = 8
NB = N // NC          # 1250 nodes per core
BLK = 128
NBLK = (NB + BLK - 1) // BLK   # 10 blocks; the last one holds 98 nodes
LAST = NB - (NBLK - 1) * BLK   # 98
EPS = 1e-5
GMAX = 8              # k-tiles per dma_gather (1024 rows = SWDGE ring cap)


def _nb_of(b):
    return min(BLK, NB - b * BLK)


# ----------------------------------------------------------------------------
# Host-side preprocessing
# ----------------------------------------------------------------------------

def _balance_nodes(wt):
    """Greedy LPT assignment of nodes to the 80 (core, block) bins so the
    per-block gather work is balanced. Returns perm: orig node -> new id."""
    nbins = NC * NBLK
    cap = np.full(nbins, BLK, np.int64)
    cap[NBLK - 1:: NBLK] = LAST
    order = np.argsort(-wt, kind="stable")
    heap = [(0, b) for b in range(nbins)]
    heapq.heapify(heap)
    fill = np.zeros(nbins, np.int64)
    perm = np.empty(N, np.int64)
    base = np.arange(nbins) // NBLK * NB + np.arange(nbins) % NBLK * BLK
    for node in order:
        while True:
            load, b = heapq.heappop(heap)
            if fill[b] < cap[b]:
                break
        perm[node] = base[b] + fill[b]
        fill[b] += 1
        if fill[b] < cap[b]:
            heapq.heappush(heap, (load + int(wt[node]), b))
    return perm


def _graph_prep(edge_index):
    """Build per-core gather index planes and dense segment-weight blocks,
    with dst-side node balancing and per-block source deduplication."""
    src = edge_index[0].astype(np.int64)
    dst = edge_index[1].astype(np.int64)
    deg = np.bincount(dst, minlength=N).astype(np.float64) + 1.0
    dinv = deg ** -0.5
    loop = np.arange(N, dtype=np.int64)
    esrc = np.concatenate([src, loop])
    edst = np.concatenate([dst, loop])
    ew = (dinv[esrc] * dinv[edst]).astype(np.float32)

    perm = _balance_nodes(deg)  # deg ~ per-dst gather row count
    psrc = perm[esrc]
    pdst = perm[edst]

    core = pdst // NB
    loc = pdst - core * NB
    blk = loc // BLK
    m = loc - blk * BLK
    gid = core * NBLK + blk
    order = np.argsort(gid, kind="stable")
    psrc, ew, m, gid = psrc[order], ew[order], m[order], gid[order]
    starts = np.searchsorted(gid, np.arange(NC * NBLK))
    ends = np.concatenate([starts[1:], [len(gid)]])

    # per-block dedup of gather sources
    uniq_lists = []
    kk = np.empty(len(gid), np.int64)
    counts = np.empty(NC * NBLK, np.int64)
    for g in range(NC * NBLK):
        s, e = starts[g], ends[g]
        u, inv = np.unique(psrc[s:e], return_inverse=True)
        uniq_lists.append(u)
        kk[s:e] = inv
        counts[g] = len(u)

    k_pad = int(np.ceil(max(counts.max(), 128) / 128.0) * 128)
    T = k_pad // 128

    wsegT = np.zeros((NC, 128, NBLK, T, BLK), np.float32)
    core_g = gid // NBLK
    blk_g = gid % NBLK
    np.add.at(wsegT, (core_g, kk % 128, blk_g, kk // 128, m), ew)
    wsegT = wsegT.astype(np.float16)

    # dma_gather index planes: slot i = t*128+p lands at out[i%128, i//128];
    # the ucode reads idxs[i] from plane[i%16, i//16], replicated across all
    # eight 16-partition groups (each Q7 core reads its own partitions).
    idx16 = np.zeros((NC, 128, NBLK, 8 * T), np.int16)
    for g in range(NC * NBLK):
        u = uniq_lists[g]
        arr = np.zeros(k_pad, np.int16)
        arr[: len(u)] = u.astype(np.int16)
        plane = arr.reshape(8 * T, 16).T          # [16, 8T]
        idx16[g // NBLK, :, g % NBLK, :] = np.tile(plane, (8, 1))
    return wsegT, idx16, k_pad, perm


def _w_moving(w):
    """[K, Nout] -> moving layout [128, Kt, Nout] fp16 (partition = K % 128)."""
    K, Nout = w.shape
    Kt = (K + 127) // 128
    out = np.zeros((128, Kt, Nout), np.float16)
    for t in range(Kt):
        rows = w[t * 128: min((t + 1) * 128, K)]
        out[: rows.shape[0], t] = rows.astype(np.float16)
    return out


def _w_stationary(w):
    """[K, M] -> stationary tiles [128, Kt, Mt, 128] fp16."""
    K, M = w.shape
    Kt = (K + 127) // 128
    Mt = (M + 127) // 128
    out = np.zeros((128, Kt, Mt, 128), np.float16)
    for t in range(Kt):
        for u in range(Mt):
            blk = w[t * 128: min((t + 1) * 128, K),
                    u * 128: min((u + 1) * 128, M)].astype(np.float16)
            out[: blk.shape[0], t, u, : blk.shape[1]] = blk
    return out


# ----------------------------------------------------------------------------
# Bass program
# ----------------------------------------------------------------------------


def _build_nc(k_pad, nontriv, use_collectives=True):
    import concourse.bacc as bacc
    import concourse.bass as bass
    import concourse.mybir as mybir
    import concourse.tile as tile
    from concourse.alu_op_type import AluOpType
    from contextlib import ExitStack

    f16 = mybir.dt.float16
    f32 = mybir.dt.float32
    f8 = mybir.dt.float8e4
    u8 = mybir.dt.uint8
    i16 = mybir.dt.int16
    AF = mybir.ActivationFunctionType
    T = k_pad // 128
    CH = [(q * GMAX, min((q + 1) * GMAX, T))
          for q in range((T + GMAX - 1) // GMAX)]
    RG = [list(range(NC))]

    nc = bacc.Bacc("TRN2", target_bir_lowering=False, debug=False,
                   num_devices=NC)

    # ---- I/O ----
    xT_d = nc.dram_tensor("xT", [F_IN, NB], f16, kind="ExternalInput")
    idx_d = nc.dram_tensor("idx16", [128, NBLK, 8 * T], i16,
                           kind="ExternalInput")
    wseg_d = nc.dram_tensor("wsegT", [128, NBLK, T, BLK], f16,
                            kind="ExternalInput")
    w_in_d = nc.dram_tensor("w_in_m", [128, 1, H], f16, kind="ExternalInput")
    w0_d = nc.dram_tensor("w0_m", [P4, 128, 2, H], f16, kind="ExternalInput")
    w12_d = nc.dram_tensor("w12_m", [2, P4, 128, 8, H], f16,
                           kind="ExternalInput")
    w1_d = nc.dram_tensor("w1_st", [128, 8, 2, 128], f16, kind="ExternalInput")
    w2_d = nc.dram_tensor("w2_st", [128, 2, 1, 128], f16, kind="ExternalInput")
    w3_d = nc.dram_tensor("w3_st", [128, 1], f16, kind="ExternalInput")
    ident_d = nc.dram_tensor("ident", [128, 128], f16, kind="ExternalInput")
    eps_d = nc.dram_tensor("eps_bc", [128, 1], f32, kind="ExternalInput")
    if nontriv["b_in"]:
        b_in_d = nc.dram_tensor("b_in_bc", [128, H], f32, kind="ExternalInput")
    if nontriv["bcat"]:
        bcat_d = nc.dram_tensor("bcat_bc", [L, 128, PH], f32,
                                kind="ExternalInput")
    if nontriv["ln"]:
        lng_d = nc.dram_tensor("lng_bc", [L, 128, PH], f32,
                               kind="ExternalInput")
        lnb_d = nc.dram_tensor("lnb_bc", [L, 128, PH], f32,
                               kind="ExternalInput")
    if nontriv["b1"]:
        b1_d = nc.dram_tensor("b1_c", [128, 2], f32, kind="ExternalInput")
    if nontriv["b2"]:
        b2_d = nc.dram_tensor("b2_c", [128, 1], f32, kind="ExternalInput")
    y_d = nc.dram_tensor("y_out", [NB], f32, kind="ExternalOutput")

    # ---- internal DRAM: AG inputs (local) and gather tables (shared) ----
    # fp8 plan: the z1 stage rows are PACKED bytes [256 fp16 | 512 fp8]:
    # the z1 columns (which feed power 1 directly and are most
    # error-sensitive) stay fp16, while [z2|z3] travel fp8e4m3 (their fp8
    # noise is carried through 2-3 further propagation hops); one u8
    # gather per chunk serves both via bitcast views. The [Az2|Az3] stage
    # is fp8 too. 256-wide tables stay fp16 (sub-512B descriptors pay a
    # 2x latency multiplier, so fp8 would not shrink their time).
    ag_in = {}
    table = {}
    tdt = {}
    for name, width, dt in [
            ("l0h0", H, f16), ("l0h1", H, f16), ("l0h2", H, f16),
            ("l1z1", 4 * H, u8), ("l1z2", 2 * H, f8), ("l1z3", H, f16),
            ("l2z1", 4 * H, u8), ("l2z2", 2 * H, f8), ("l2z3", H, f16)]:
        tdt[name] = dt
        ag_in[name] = nc.dram_tensor(f"agin_{name}", [NB, width], dt)
        table[name] = nc.dram_tensor(f"tab_{name}", [N, width], dt,
                                     addr_space="Shared")

    with tile.TileContext(nc) as tc, ExitStack() as ctx:
        const = ctx.enter_context(tc.tile_pool(name="const", bufs=1))
        work = ctx.enter_context(tc.tile_pool(name="work", bufs=2))
        big = ctx.enter_context(tc.tile_pool(name="big", bufs=1))
        gpool = ctx.enter_context(tc.tile_pool(name="gpool", bufs=3))
        one = ctx.enter_context(tc.tile_pool(name="one", bufs=1))
        psum = ctx.enter_context(tc.tile_pool(name="psum", bufs=6,
                                              space="PSUM"))
        pstr = ctx.enter_context(tc.tile_pool(name="pstr", bufs=2,
                                              space="PSUM"))

        # ---- persistent SBUF constants (h0 operands first) ----
        xT_sb = const.tile([F_IN, NB], f16, tag="xT")
        nc.sync.dma_start(out=xT_sb[:], in_=xT_d[:])
        w_in_sb = const.tile([128, 1, H], f16, tag="w_in")
        nc.sync.dma_start(out=w_in_sb[:], in_=w_in_d[:])
        ident_sb = const.tile([128, 128], f16, tag="ident")
        nc.sync.dma_start(out=ident_sb[:], in_=ident_d[:])
        eps_sb = const.tile([128, 1], f32, tag="eps")
        nc.sync.dma_start(out=eps_sb[:], in_=eps_d[:])
        zero_sb = const.tile([128, 1], f32, tag="zero")
        nc.vector.memset(zero_sb[:], 0.0)
        wseg_sb = const.tile([128, NBLK, T, BLK], f16, tag="wseg")
        nc.sync.dma_start(out=wseg_sb[:], in_=wseg_d[:])
        idx_sb = const.tile([128, NBLK, 8 * T], i16, tag="idx")
        nc.sync.dma_start(out=idx_sb[:], in_=idx_d[:])
        w0_sb = const.tile([128, P4, 2, H], f16, tag="w0")
        for p in range(P4):
            nc.sync.dma_start(out=w0_sb[:, p, :, :], in_=w0_d[p])
        w1_sb = const.tile([128, 8, 2, 128], f16, tag="w1")
        nc.sync.dma_start(out=w1_sb[:], in_=w1_d[:])
        w2_sb = const.tile([128, 2, 1, 128], f16, tag="w2")
        nc.sync.dma_start(out=w2_sb[:], in_=w2_d[:])
        w3_sb = const.tile([128, 1], f16, tag="w3")
        nc.sync.dma_start(out=w3_sb[:], in_=w3_d[:])
        if nontriv["b_in"]:
            b_in_sb = const.tile([128, H], f32, tag="b_in")
            nc.sync.dma_start(out=b_in_sb[:], in_=b_in_d[:])
        if nontriv["bcat"]:
            bcat_sb = const.tile([128, L, PH], f32, tag="bcat")
            for i in range(L):
                nc.sync.dma_start(out=bcat_sb[:, i, :], in_=bcat_d[i])
        if nontriv["ln"]:
            lng_sb = const.tile([128, L, PH], f32, tag="lng")
            lnb_sb = const.tile([128, L, PH], f32, tag="lnb")
            for i in range(L):
                nc.sync.dma_start(out=lng_sb[:, i, :], in_=lng_d[i])
                nc.sync.dma_start(out=lnb_sb[:, i, :], in_=lnb_d[i])
        if nontriv["b1"]:
            b1_sb = const.tile([128, 2], f32, tag="b1")
            nc.sync.dma_start(out=b1_sb[:], in_=b1_d[:])
        if nontriv["b2"]:
            b2_sb = const.tile([128, 1], f32, tag="b2")
            nc.sync.dma_start(out=b2_sb[:], in_=b2_d[:])

        # persistent activations. During layer 0, hT[:, 2p:2p+2, :] holds the
        # feature-major transpose of A^p h (the hops' projection operands);
        # after each layernorm it holds the feature-major layer output.
        hT = big.tile([128, 8, NB], f16, tag="hT")
        hcat = big.tile([128, NBLK, PH], f16, tag="hcat")
        # node-major staging slabs for the next AllGather input (batched
        # write): fp16 for 256-wide slabs, fp8 for the wide (768/512) ones
        stg = big.tile([128, NBLK, 2 * H], f16, tag="stg")
        stg8 = big.tile([128, NBLK, 2 * H], f8, tag="stg8")

        def zb(nb):
            return zero_sb[:nb, 0:1]

        def allgather(name):
            """Halo exchange ag_in[name] -> table[name]. With collectives off
            (cost-model timing builds, which can't model ncfw), stand in a
            local DMA with the same per-core HBM write volume."""
            if use_collectives:
                nc.gpsimd.collective_compute(
                    "AllGather", AluOpType.bypass, replica_groups=RG,
                    ins=[ag_in[name][:]], outs=[table[name][:]],
                )
            else:
                for c in range(2):
                    nc.sync.dma_start(
                        out=table[name][c * NB:(c + 1) * NB, :],
                        in_=ag_in[name][:])

        def stage_flush(name, width, slab):
            """Write the staged [NB, width] slab to ag_in[name] in 2 DMAs."""
            full = (NBLK - 1) * BLK  # 1152
            nc.sync.dma_start(
                out=ag_in[name][:full, :].rearrange("(a p) f -> p a f", p=128),
                in_=slab[:, : NBLK - 1, :width])
            nc.sync.dma_start(
                out=ag_in[name][full:, :],
                in_=slab[:LAST, NBLK - 1, :width])

        def transposes_to(dst_ap3, src_ap2, nb, nkt):
            """dst[128, nkt, nb] (feature-major) = per-k-tile transposes of
            src[nb, nkt*128] via PE, staged through one PSUM bank and copied
            out with a single (strided) DVE copy."""
            pst = pstr.tile([128, nkt, 128], f16, tag="tr")
            for kt in range(nkt):
                nc.tensor.transpose(pst[:, kt, :nb],
                                    src_ap2[:nb, kt * 128:(kt + 1) * 128],
                                    ident_sb[:nb, :nb])
            nc.vector.tensor_copy(dst_ap3, pst[:, :, :nb])

        def seg_psums(tabs, b):
            """Propagation block b: dma_gather the block's (deduplicated)
            source rows of each table in `tabs` into k-tiles, then
            segment-matmul on PE. Each entry of `tabs` is (tab, views)
            where views = [(byte_lo, dtype, ncols)] reinterprets byte
            ranges of the gathered rows (for the packed fp16|fp8 z1
            table); plain tables pass one full-width view. Psum pieces
            are aligned to view boundaries; returns [(c0, cw, psum_tile)]
            over the concatenated column space."""
            outs = []
            srcs = []          # (tab_idx, view) per psum piece
            base = 0
            for ti, (tab, views) in enumerate(tabs):
                for (blo, vdt, ncols) in views:
                    c0 = 0
                    while c0 < ncols:
                        cw = min(512, ncols - c0)
                        ps = psum.tile([128, 512], f32, tag="mm",
                                       name="ps_seg")
                        outs.append((base + c0, cw, ps))
                        srcs.append((ti, blo, vdt, c0))
                        c0 += cw
                    base += ncols
            for (t0, t1) in CH:
                gts = []
                for (tab, views) in tabs:
                    width = tab.shape[1]
                    gt = gpool.tile([128, t1 - t0, width], tab.dtype,
                                    tag="gath")
                    nc.gpsimd.dma_gather(
                        out_ap=gt[:],
                        in_ap=tab[:],
                        idxs_ap=idx_sb[:, b, t0 * 8: t1 * 8],
                        num_idxs=(t1 - t0) * 128,
                        num_idxs_reg=(t1 - t0) * 128,
                        elem_size=width,
                    )
                    gts.append((gt, tab.dtype))
                for (_, cw, ps), (ti, blo, vdt, tc0) in zip(outs, srcs):
                    gt, gdt = gts[ti]
                    gsz = mybir.dt.size(gdt)
                    lo = (blo + tc0 * mybir.dt.size(vdt)) // gsz
                    ncnt = cw * mybir.dt.size(vdt) // gsz
                    for t in range(t0, t1):
                        rhs = gt[:, t - t0, lo: lo + ncnt]
                        if vdt != gdt:
                            rhs = rhs.bitcast(vdt)
                        nc.tensor.matmul(
                            ps[:, :cw],
                            wseg_sb[:, b, t, :],
                            rhs,
                            start=(t == 0),
                            stop=(t == T - 1),
                        )
            return outs

        def ln_pass1(layer, b):
            """hcat[:, b] -> (+bias) -> bn stats -> mv (mu, var, std, rstd,
            -mu*rstd). Issued inside the last hop/projection of the layer so
            the Sqrts run while ACT is otherwise idle (one table load)."""
            hc = hcat[:, b, :]
            if nontriv["bcat"]:
                nc.vector.tensor_tensor(hc, hc, bcat_sb[:, layer, :],
                                        AluOpType.add)
            st = work.tile([128, 12], f32, tag="bnst", name="st")
            nc.vector.bn_stats(st[:, 0:6], hcat[:, b, 0:512])
            nc.vector.bn_stats(st[:, 6:12], hcat[:, b, 512:1024])
            mv = work.tile([128, 6], f32, tag=f"bnmv{b}", name="mv")
            nc.vector.bn_aggr(mv[:, 0:2], st[:])
            nc.scalar.activation(mv[:, 2:3], mv[:, 1:2], AF.Sqrt,
                                 bias=eps_sb[:, 0:1])
            nc.vector.reciprocal(mv[:, 3:4], mv[:, 2:3])
            # mv[:,4] = -mu * rstd, the fused-activation bias
            nc.vector.tensor_scalar(mv[:, 4:5], mv[:, 0:1], mv[:, 3:4],
                                    -1.0, AluOpType.mult, AluOpType.mult)
            return mv

        def ln_pass2(layer, mvs, next_blk=None):
            """normalize+gelu (one fused ACT op per block) -> hT transposes,
            interleaving `next_blk(b)` (next-layer projections / MLP chunks)
            so PE never drains between blocks."""
            for b in range(NBLK):
                nb = _nb_of(b)
                mv = mvs[b]
                gl = work.tile([128, PH], f16, tag="gel")
                if nontriv["ln"]:
                    xn = one.tile([128, PH], f32, tag="xn")
                    nc.vector.tensor_scalar(
                        xn[:], hcat[:, b, :], mv[:, 0:1], mv[:, 3:4],
                        AluOpType.subtract, AluOpType.mult,
                    )
                    nc.vector.tensor_tensor(xn[:], xn[:],
                                            lng_sb[:, layer, :],
                                            AluOpType.mult)
                    nc.vector.tensor_tensor(xn[:], xn[:],
                                            lnb_sb[:, layer, :],
                                            AluOpType.add)
                    nc.scalar.activation(gl[:], xn[:], AF.Gelu, bias=zb(128))
                else:
                    # gelu((x - mu) * rstd) == gelu(x * rstd + (-mu * rstd))
                    nc.scalar.activation(gl[:], hcat[:, b, :], AF.Gelu,
                                         bias=mv[:, 4:5], scale=mv[:, 3:4])
                transposes_to(hT[:, :, b * BLK: b * BLK + nb], gl, nb, 8)
                if next_blk is not None:
                    next_blk(b)

        # ================= stage 0: h0 = gelu(x @ w_in + b_in) =============
        for b in range(NBLK):
            nb = _nb_of(b)
            ps = psum.tile([128, 512], f32, tag="mm")
            nc.tensor.matmul(ps[:nb, :H],
                             xT_sb[:, b * BLK: b * BLK + nb],
                             w_in_sb[:F_IN, 0, :], start=True, stop=True)
            if nontriv["b_in"]:
                tmp = work.tile([128, 512], f32, tag="btmp")
                nc.vector.tensor_tensor(tmp[:nb, :H], ps[:nb, :H],
                                        b_in_sb[:nb, :], AluOpType.add)
                nc.scalar.activation(stg[:nb, b, :H], tmp[:nb, :H], AF.Gelu,
                                     bias=zb(nb))
            else:
                nc.scalar.activation(stg[:nb, b, :H], ps[:nb, :H], AF.Gelu,
                                     bias=zb(nb))
            transposes_to(hT[:, 0:2, b * BLK: b * BLK + nb],
                          stg[:, b, :H], nb, 2)
        stage_flush("l0h0", H, stg)
        allgather("l0h0")

        # ================= layer 0: propagate-then-project =================
        mvs = [None] * NBLK

        def l0_project(p):
            """hcat[:, b, p*H:(p+1)*H] = h_p @ mh_w0[p] from hT[:, 2p:2p+2].
            The p=3 pass completes hcat, so it chains each block's LN
            stats (ln_pass1) right behind its projection."""
            for b in range(NBLK):
                nb = _nb_of(b)
                ps = psum.tile([128, 512], f32, tag="mm")
                for kt in range(2):
                    nc.tensor.matmul(ps[:nb, :H],
                                     hT[:, 2 * p + kt, b * BLK: b * BLK + nb],
                                     w0_sb[:, p, kt, :],
                                     start=(kt == 0), stop=(kt == 1))
                nc.vector.tensor_copy(hcat[:nb, b, p * H:(p + 1) * H],
                                      ps[:nb, :H])

        l0_project(0)
        hops = [("l0h0", "l0h1"), ("l0h1", "l0h2"), ("l0h2", None)]
        for p, (tin, tout) in enumerate(hops, start=1):
            for b in range(NBLK):
                nb = _nb_of(b)
                (_, _, ps), = seg_psums([(table[tin], [(0, f16, H)])], b)
                if tout is not None:
                    nc.vector.tensor_copy(stg[:, b, :H], ps[:, :H])
                    transposes_to(hT[:, 2 * p: 2 * p + 2, b * BLK: b * BLK + nb],
                                  stg[:, b, :H], nb, 2)
                else:
                    sg = work.tile([128, H], f16, tag="sg")
                    nc.vector.tensor_copy(sg[:], ps[:, :H])
                    transposes_to(hT[:, 2 * p: 2 * p + 2, b * BLK: b * BLK + nb],
                                  sg[:, :H], nb, 2)
            if tout is not None:
                stage_flush(tout, H, stg)
                allgather(tout)
            l0_project(p)

        # ================= layers 1-2: project-first ======================
        def make_proj(layer, w12_sb):
            def proj(b):
                """MixHop projections for `layer`: p=0 -> hcat, p=1 -> fp16
                z1 staging, p=2,3 -> fp8 staging (AG input)."""
                nb = _nb_of(b)
                for p in range(P4):
                    ps = psum.tile([128, 512], f32, tag="mm")
                    for kt in range(8):
                        nc.tensor.matmul(ps[:nb, :H],
                                         hT[:, kt, b * BLK: b * BLK + nb],
                                         w12_sb[:, p, kt, :],
                                         start=(kt == 0), stop=(kt == 7))
                    if p == 0:
                        nc.vector.tensor_copy(hcat[:nb, b, 0:H], ps[:nb, :H])
                    elif p == 1:
                        nc.vector.tensor_copy(stg[:nb, b, :H], ps[:nb, :H])
                    else:
                        nc.vector.tensor_copy(
                            stg8[:nb, b, (p - 2) * H: (p - 1) * H],
                            ps[:nb, :H])
            return proj

        def z1_flush_ag(zn):
            # packed z1 flush: fp16 z1 bytes [0:512), fp8 [z2|z3] [512:1024)
            agz = ag_in[zn]
            full = (NBLK - 1) * BLK
            bulk = agz[:full, :].rearrange("(a p) f -> p a f", p=128)
            nc.sync.dma_start(out=bulk[:, :, 0: 2 * H].bitcast(f16),
                              in_=stg[:, : NBLK - 1, :H])
            nc.sync.dma_start(out=bulk[:, :, 2 * H: 4 * H].bitcast(f8),
                              in_=stg8[:, : NBLK - 1, : 2 * H])
            nc.sync.dma_start(out=agz[full:, 0: 2 * H].bitcast(f16),
                              in_=stg[:LAST, NBLK - 1, :H])
            nc.sync.dma_start(out=agz[full:, 2 * H: 4 * H].bitcast(f8),
                              in_=stg8[:LAST, NBLK - 1, : 2 * H])
            allgather(zn)

        def run_hops(layer):
            """The three propagation hops of a MixHop layer; the last hop
            chains per-block LN stats."""
            zname = [f"l{layer}z1", f"l{layer}z2", f"l{layer}z3"]
            hop_tabs = [[(table[zname[0]],
                          [(0, f16, H), (2 * H, f8, 2 * H)])],
                        [(table[zname[1]], [(0, f8, 2 * H)])],
                        [(table[zname[2]], [(0, f16, H)])]]
            for hop in range(3):
                width = (3 - hop) * H
                tout = zname[hop + 1] if hop < 2 else None
                for b in range(NBLK):
                    nb = _nb_of(b)
                    pieces = seg_psums(hop_tabs[hop], b)
                    # first H columns are this hop's power output
                    nc.vector.tensor_copy(
                        hcat[:nb, b, (hop + 1) * H:(hop + 2) * H],
                        pieces[0][2][:nb, :H])
                    if tout is not None:
                        oslab = stg8 if tdt[tout] == f8 else stg
                        for (c0, cw, ps) in pieces:
                            if c0 + cw <= H:
                                continue
                            lo = max(H, c0)
                            nc.vector.tensor_copy(
                                oslab[:, b, lo - H: c0 + cw - H],
                                ps[:, lo - c0: cw])
                if tout is not None:
                    stage_flush(tout, width - H,
                                stg8 if tdt[tout] == f8 else stg)
                    allgather(tout)

        # layer 1 projections interleave with layer 0's LN pass 2
        w12_sb1 = const.tile([128, P4, 8, H], f16, tag="w12a")
        for p in range(P4):
            nc.sync.dma_start(out=w12_sb1[:, p, :, :], in_=w12_d[0, p])
        for b in range(NBLK):
            mvs[b] = ln_pass1(0, b)
        ln_pass2(0, mvs)
        proj1 = make_proj(1, w12_sb1)
        for b in range(NBLK):
            proj1(b)
        z1_flush_ag("l1z1")
        run_hops(1)

        w12_sb2 = const.tile([128, P4, 8, H], f16, tag="w12b")
        for p in range(P4):
            nc.sync.dma_start(out=w12_sb2[:, p, :, :], in_=w12_d[1, p])
        for b in range(NBLK):
            mvs[b] = ln_pass1(1, b)
        ln_pass2(1, mvs)
        proj2 = make_proj(2, w12_sb2)
        for b in range(NBLK):
            proj2(b)
        z1_flush_ag("l2z1")
        run_hops(2)

        # ============ final MLP, interleaved with layer 2's LN ============
        m1T = big.tile([128, 2, NB], f16, tag="stg", name="m1T")
        m2T = big.tile([128, NB], f16, tag="hcat", name="m2T")
        chunks = [(c, min(512, NB - c)) for c in range(0, NB, 512)]

        def mlp_chunk(ci):
            c0, cw = chunks[ci]
            for mt in range(2):
                ps = psum.tile([128, 512], f32, tag="mm")
                for kt in range(8):
                    nc.tensor.matmul(ps[:, :cw], w1_sb[:, kt, mt, :],
                                     hT[:, kt, c0:c0 + cw],
                                     start=(kt == 0), stop=(kt == 7))
                bias = b1_sb[:, mt:mt + 1] if nontriv["b1"] else zb(128)
                nc.scalar.activation(m1T[:, mt, c0:c0 + cw], ps[:, :cw],
                                     AF.Gelu, bias=bias)
            ps = psum.tile([128, 512], f32, tag="mm")
            for kt in range(2):
                nc.tensor.matmul(ps[:, :cw], w2_sb[:, kt, 0, :],
                                 m1T[:, kt, c0:c0 + cw],
                                 start=(kt == 0), stop=(kt == 1))
            bias = b2_sb[:, 0:1] if nontriv["b2"] else zb(128)
            nc.scalar.activation(m2T[:, c0:c0 + cw], ps[:, :cw],
                                 AF.Gelu, bias=bias)

        def mlp_blk(b):
            # chunk ci spans hT columns [ci*512, ci*512+512) -> ready once
            # blocks 0..b cover them
            if b == 3:
                mlp_chunk(0)
            elif b == 7:
                mlp_chunk(1)
            elif b == 9:
                mlp_chunk(2)

        for b in range(NBLK):
            mvs[b] = ln_pass1(2, b)
        ln_pass2(2, mvs)
        for ci in range(3):
            mlp_chunk(ci)
        ysb = big.tile([1, NB], f32, tag="ysb", name="ysb")
        for (c0, cw) in chunks:
            ps = psum.tile([128, 512], f32, tag="mm")
            nc.tensor.matmul(ps[:1, :cw], w3_sb[:, :1], m2T[:, c0:c0 + cw],
                             start=True, stop=True)
            nc.vector.tensor_copy(ysb[:1, c0:c0 + cw], ps[:1, :cw])
        nc.sync.dma_start(out=y_d[:], in_=ysb[:1, :])

    nc.compile()
    return nc


# ----------------------------------------------------------------------------
# Public entry point
# ----------------------------------------------------------------------------

_CACHE = {}


def _prep_inputs(inputs):
    x = np.asarray(inputs["x"], np.float32)
    edge_index = np.asarray(inputs["edge_index"])
    wsegT, idx16, k_pad, perm = _graph_prep(edge_index)

    b3 = np.asarray(inputs["b3"], np.float32)
    nontriv = {
        "b_in": bool(np.any(inputs["b_in"])),
        "bcat": bool(np.any(inputs["mh_b0"]) or np.any(inputs["mh_b12"])),
        "ln": not (np.allclose(np.asarray(inputs["ln_g"]), 1.0)
                   and not np.any(inputs["ln_b"])),
        "b1": bool(np.any(inputs["b1"])),
        "b2": bool(np.any(inputs["b2"])),
    }

    shared = {
        "w_in_m": _w_moving(np.asarray(inputs["w_in"], np.float32)),
        "w0_m": np.stack([_w_moving(np.asarray(inputs["mh_w0"][p], np.float32))
                          for p in range(P4)]),
        "w12_m": np.stack([
            np.stack([_w_moving(np.asarray(inputs["mh_w12"][l, p], np.float32))
                      for p in range(P4)])
            for l in range(2)]),
        "w1_st": _w_stationary(np.asarray(inputs["w1"], np.float32)),
        "w2_st": _w_stationary(np.asarray(inputs["w2"], np.float32)),
        "w3_st": np.asarray(inputs["w3"], np.float32).astype(np.float16),
        "ident": np.eye(128, dtype=np.float16),
        "eps_bc": np.full((128, 1), EPS, np.float32),
    }
    if nontriv["b_in"]:
        shared["b_in_bc"] = np.tile(np.asarray(inputs["b_in"], np.float32),
                                    (128, 1))
    if nontriv["bcat"]:
        bcat = np.zeros((L, PH), np.float32)
        bcat[0] = np.asarray(inputs["mh_b0"], np.float32).reshape(-1)
        bcat[1] = np.asarray(inputs["mh_b12"], np.float32)[0].reshape(-1)
        bcat[2] = np.asarray(inputs["mh_b12"], np.float32)[1].reshape(-1)
        shared["bcat_bc"] = np.ascontiguousarray(
            np.broadcast_to(bcat[:, None, :], (L, 128, PH)))
    if nontriv["ln"]:
        shared["lng_bc"] = np.ascontiguousarray(np.broadcast_to(
            np.asarray(inputs["ln_g"], np.float32)[:, None, :], (L, 128, PH)))
        shared["lnb_bc"] = np.ascontiguousarray(np.broadcast_to(
            np.asarray(inputs["ln_b"], np.float32)[:, None, :], (L, 128, PH)))
    if nontriv["b1"]:
        shared["b1_c"] = np.ascontiguousarray(
            np.asarray(inputs["b1"], np.float32).reshape(2, 128).T)
    if nontriv["b2"]:
        shared["b2_c"] = np.asarray(inputs["b2"], np.float32).reshape(128, 1)

    xp = x[np.argsort(perm)]  # xp[newid] = x[orig]
    in_maps = []
    for c in range(NC):
        m = dict(shared)
        m["xT"] = np.ascontiguousarray(
            xp[c * NB:(c + 1) * NB].T.astype(np.float16))
        m["idx16"] = np.ascontiguousarray(idx16[c])
        m["wsegT"] = np.ascontiguousarray(wsegT[c])
        in_maps.append(m)
    return in_maps, k_pad, nontriv, b3, perm


def _run(inputs, trace=False, **kwargs):
    from concourse.bass_utils import run_bass_kernel_spmd

    in_maps, k_pad, nontriv, b3, perm = _prep_inputs(inputs)
    key = (k_pad, tuple(sorted(nontriv.items())))
    if key not in _CACHE:
        _CACHE[key] = _build_nc(k_pad, nontriv)
    nc = _CACHE[key]
    res = run_bass_kernel_spmd(nc, in_maps, list(range(NC)), trace=trace,
                               **kwargs)
    ycat = np.concatenate([res.results[c]["y_out"] for c in range(NC)])
    y = ycat[perm].astype(np.float32) + b3.reshape(-1)[0]
    return y, res


def kernel(**inputs) -> np.ndarray:
    y, _ = _run(inputs, trace=False)
    return y


# revision 21
# speedup vs baseline: 1.0744x; 1.0609x over previous
"""MixHopVolatilityNet Trainium2 kernel (8 NeuronCores, SPMD).

Strategy (graph/data parallel, per sharding hint):
 - Nodes partitioned across 8 cores (1250 each) via a degree-balanced
   permutation; each core owns the destination side of every propagation
   for its nodes. Weights replicated.
 - Halo exchange: after each hop every core AllGathers its 1250-row slab
   into the next full [10000, F] feature table (on-chip ncfw collective).
 - Every hop runs as gather + segment-matmul per 128-dst-node block: the
   (deduplicated, per-block) source rows of the replicated table are
   batch-gathered into SBUF k-tiles with gpsimd.dma_gather (one Q7
   instruction per <=1024 rows, ~1us desc-gen each instead of the ~17us
   of per-128-row indirect DMAs), then PE accumulates
   wsegT[k_slot, dst]^T @ gathered[k_slot, :] over the k-tiles.
 - Layer 0 propagates h directly (propagate-then-project, 3x256-wide
   hops). Layers 1-2 project first (out_p = A^p (h @ W_p) + b_p),
   batching powers into [z1|z2|z3] so hops are 768/512/256 wide.
 - Per-hop staging of the AllGather input slab is batched into one SBUF
   tile and written with 2 DMAs (not 10) to cut HWDGE/queue overhead.
 - All matmul operands fp16 (PSUM accumulates fp32); layernorm (two-pass,
   bn_stats/bn_aggr) in fp32; erf-gelu via the ACT Gelu LUT.
"""

import heapq
import sys

import numpy as np

sys.path.insert(0, "/opt/trn_rl_repo")

# ---- problem constants (hardcoded per contract) ----
N = 10000
E = 160000
F_IN = 84
H = 256
P4 = 4
L = 3
PH = P4 * H  # 1024
NC = 8
NB = N // NC          # 1250 nodes per core
BLK = 128
NBLK = (NB + BLK - 1) // BLK   # 10 blocks; the last one holds 98 nodes
LAST = NB - (NBLK - 1) * BLK   # 98
EPS = 1e-5
GMAX = 8              # k-tiles per dma_gather (1024 rows = SWDGE ring cap)


def _nb_of(b):
    return min(BLK, NB - b * BLK)


# ----------------------------------------------------------------------------
# Host-side preprocessing
# ----------------------------------------------------------------------------

def _balance_nodes(wt):
    """Greedy LPT assignment of nodes to the 80 (core, block) bins so the
    per-block gather work is balanced. Returns perm: orig node -> new id."""
    nbins = NC * NBLK
    cap = np.full(nbins, BLK, np.int64)
    cap[NBLK - 1:: NBLK] = LAST
    order = np.argsort(-wt, kind="stable")
    heap = [(0, b) for b in range(nbins)]
    heapq.heapify(heap)
    fill = np.zeros(nbins, np.int64)
    perm = np.empty(N, np.int64)
    base = np.arange(nbins) // NBLK * NB + np.arange(nbins) % NBLK * BLK
    for node in order:
        while True:
            load, b = heapq.heappop(heap)
            if fill[b] < cap[b]:
                break
        perm[node] = base[b] + fill[b]
        fill[b] += 1
        if fill[b] < cap[b]:
            heapq.heappush(heap, (load + int(wt[node]), b))
    return perm


def _graph_prep(edge_index):
    """Build per-core gather index planes and dense segment-weight blocks,
    with dst-side node balancing and per-block source deduplication."""
    src = edge_index[0].astype(np.int64)
    dst = edge_index[1].astype(np.int64)
    deg = np.bincount(dst, minlength=N).astype(np.float64) + 1.0
    dinv = deg ** -0.5
    loop = np.arange(N, dtype=np.int64)
    esrc = np.concatenate([src, loop])
    edst = np.concatenate([dst, loop])
    ew = (dinv[esrc] * dinv[edst]).astype(np.float32)

    perm = _balance_nodes(deg)  # deg ~ per-dst gather row count
    psrc = perm[esrc]
    pdst = perm[edst]

    core = pdst // NB
    loc = pdst - core * NB
    blk = loc // BLK
    m = loc - blk * BLK
    gid = core * NBLK + blk
    order = np.argsort(gid, kind="stable")
    psrc, ew, m, gid = psrc[order], ew[order], m[order], gid[order]
    starts = np.searchsorted(gid, np.arange(NC * NBLK))
    ends = np.concatenate([starts[1:], [len(gid)]])

    # per-block dedup of gather sources
    uniq_lists = []
    kk = np.empty(len(gid), np.int64)
    counts = np.empty(NC * NBLK, np.int64)
    for g in range(NC * NBLK):
        s, e = starts[g], ends[g]
        u, inv = np.unique(psrc[s:e], return_inverse=True)
        uniq_lists.append(u)
        kk[s:e] = inv
        counts[g] = len(u)

    k_pad = int(np.ceil(max(counts.max(), 128) / 128.0) * 128)
    T = k_pad // 128

    wsegT = np.zeros((NC, 128, NBLK, T, BLK), np.float32)
    core_g = gid // NBLK
    blk_g = gid % NBLK
    np.add.at(wsegT, (core_g, kk % 128, blk_g, kk // 128, m), ew)
    import ml_dtypes
    wseg8 = wsegT.astype(ml_dtypes.float8_e4m3fn)
    wsegT = wsegT.astype(np.float16)

    # dma_gather index planes: slot i = t*128+p lands at out[i%128, i//128];
    # the ucode reads idxs[i] from plane[i%16, i//16], replicated across all
    # eight 16-partition groups (each Q7 core reads its own partitions).
    idx16 = np.zeros((NC, 128, NBLK, 8 * T), np.int16)
    for g in range(NC * NBLK):
        u = uniq_lists[g]
        arr = np.zeros(k_pad, np.int16)
        arr[: len(u)] = u.astype(np.int16)
        plane = arr.reshape(8 * T, 16).T          # [16, 8T]
        idx16[g // NBLK, :, g % NBLK, :] = np.tile(plane, (8, 1))
    return wsegT, wseg8, idx16, k_pad, perm


def _w_moving(w):
    """[K, Nout] -> moving layout [128, Kt, Nout] fp16 (partition = K % 128)."""
    K, Nout = w.shape
    Kt = (K + 127) // 128
    out = np.zeros((128, Kt, Nout), np.float16)
    for t in range(Kt):
        rows = w[t * 128: min((t + 1) * 128, K)]
        out[: rows.shape[0], t] = rows.astype(np.float16)
    return out


def _w_stationary(w):
    """[K, M] -> stationary tiles [128, Kt, Mt, 128] fp16."""
    K, M = w.shape
    Kt = (K + 127) // 128
    Mt = (M + 127) // 128
    out = np.zeros((128, Kt, Mt, 128), np.float16)
    for t in range(Kt):
        for u in range(Mt):
            blk = w[t * 128: min((t + 1) * 128, K),
                    u * 128: min((u + 1) * 128, M)].astype(np.float16)
            out[: blk.shape[0], t, u, : blk.shape[1]] = blk
    return out


# ----------------------------------------------------------------------------
# Bass program
# ----------------------------------------------------------------------------


def _build_nc(k_pad, nontriv, use_collectives=True):
    import concourse.bacc as bacc
    import concourse.bass as bass
    import concourse.mybir as mybir
    import concourse.tile as tile
    from concourse.alu_op_type import AluOpType
    from contextlib import ExitStack

    f16 = mybir.dt.float16
    f32 = mybir.dt.float32
    f8 = mybir.dt.float8e4
    u8 = mybir.dt.uint8
    i16 = mybir.dt.int16
    AF = mybir.ActivationFunctionType
    T = k_pad // 128
    CH = [(q * GMAX, min((q + 1) * GMAX, T))
          for q in range((T + GMAX - 1) // GMAX)]
    RG = [list(range(NC))]

    nc = bacc.Bacc("TRN2", target_bir_lowering=False, debug=False,
                   num_devices=NC)

    # ---- I/O ----
    xT_d = nc.dram_tensor("xT", [F_IN, NB], f16, kind="ExternalInput")
    idx_d = nc.dram_tensor("idx16", [128, NBLK, 8 * T], i16,
                           kind="ExternalInput")
    wseg_d = nc.dram_tensor("wsegT", [128, NBLK, T, BLK], f16,
                            kind="ExternalInput")
    wseg8_d = nc.dram_tensor("wseg8", [128, NBLK, T, BLK], f8,
                             kind="ExternalInput")
    w_in_d = nc.dram_tensor("w_in_m", [128, 1, H], f16, kind="ExternalInput")
    w0_d = nc.dram_tensor("w0_m", [P4, 128, 2, H], f16, kind="ExternalInput")
    w12_d = nc.dram_tensor("w12_m", [2, P4, 128, 8, H], f16,
                           kind="ExternalInput")
    w1_d = nc.dram_tensor("w1_st", [128, 8, 2, 128], f16, kind="ExternalInput")
    w2_d = nc.dram_tensor("w2_st", [128, 2, 1, 128], f16, kind="ExternalInput")
    w3_d = nc.dram_tensor("w3_st", [128, 1], f16, kind="ExternalInput")
    ident_d = nc.dram_tensor("ident", [128, 128], f16, kind="ExternalInput")
    eps_d = nc.dram_tensor("eps_bc", [128, 1], f32, kind="ExternalInput")
    if nontriv["b_in"]:
        b_in_d = nc.dram_tensor("b_in_bc", [128, H], f32, kind="ExternalInput")
    if nontriv["bcat"]:
        bcat_d = nc.dram_tensor("bcat_bc", [L, 128, PH], f32,
                                kind="ExternalInput")
    if nontriv["ln"]:
        lng_d = nc.dram_tensor("lng_bc", [L, 128, PH], f32,
                               kind="ExternalInput")
        lnb_d = nc.dram_tensor("lnb_bc", [L, 128, PH], f32,
                               kind="ExternalInput")
    if nontriv["b1"]:
        b1_d = nc.dram_tensor("b1_c", [128, 2], f32, kind="ExternalInput")
    if nontriv["b2"]:
        b2_d = nc.dram_tensor("b2_c", [128, 1], f32, kind="ExternalInput")
    y_d = nc.dram_tensor("y_out", [NB], f32, kind="ExternalOutput")

    # ---- internal DRAM: AG inputs (local) and gather tables (shared) ----
    # fp8 plan: the z1 stage rows are PACKED bytes [256 fp16 | 512 fp8]:
    # the z1 columns (which feed power 1 directly and are most
    # error-sensitive) stay fp16, while [z2|z3] travel fp8e4m3 (their fp8
    # noise is carried through 2-3 further propagation hops); one u8
    # gather per chunk serves both via bitcast views. The [Az2|Az3] stage
    # is fp8 too. 256-wide tables stay fp16 (sub-512B descriptors pay a
    # 2x latency multiplier, so fp8 would not shrink their time).
    ag_in = {}
    table = {}
    tdt = {}
    for name, width, dt in [
            ("l0h0", H, f16), ("l0h1", H, f16), ("l0h2", H, f16),
            ("l1z1", 4 * H, u8), ("l1z2", 2 * H, f8), ("l1z3", H, f16),
            ("l2z1", 4 * H, u8), ("l2z2", 2 * H, f8), ("l2z3", H, f16)]:
        tdt[name] = dt
        ag_in[name] = nc.dram_tensor(f"agin_{name}", [NB, width], dt)
        table[name] = nc.dram_tensor(f"tab_{name}", [N, width], dt,
                                     addr_space="Shared")

    with tile.TileContext(nc) as tc, ExitStack() as ctx:
        const = ctx.enter_context(tc.tile_pool(name="const", bufs=1))
        work = ctx.enter_context(tc.tile_pool(name="work", bufs=2))
        big = ctx.enter_context(tc.tile_pool(name="big", bufs=1))
        gpool = ctx.enter_context(tc.tile_pool(name="gpool", bufs=3))
        one = ctx.enter_context(tc.tile_pool(name="one", bufs=1))
        psum = ctx.enter_context(tc.tile_pool(name="psum", bufs=6,
                                              space="PSUM"))
        pstr = ctx.enter_context(tc.tile_pool(name="pstr", bufs=2,
                                              space="PSUM"))

        # ---- persistent SBUF constants (h0 operands first) ----
        xT_sb = const.tile([F_IN, NB], f16, tag="xT")
        nc.sync.dma_start(out=xT_sb[:], in_=xT_d[:])
        w_in_sb = const.tile([128, 1, H], f16, tag="w_in")
        nc.sync.dma_start(out=w_in_sb[:], in_=w_in_d[:])
        ident_sb = const.tile([128, 128], f16, tag="ident")
        nc.sync.dma_start(out=ident_sb[:], in_=ident_d[:])
        eps_sb = const.tile([128, 1], f32, tag="eps")
        nc.sync.dma_start(out=eps_sb[:], in_=eps_d[:])
        zero_sb = const.tile([128, 1], f32, tag="zero")
        nc.vector.memset(zero_sb[:], 0.0)
        wseg_sb = const.tile([128, NBLK, T, BLK], f16, tag="wseg")
        nc.sync.dma_start(out=wseg_sb[:], in_=wseg_d[:])
        wseg8_sb = const.tile([128, NBLK, T, BLK], f8, tag="wseg8")
        nc.sync.dma_start(out=wseg8_sb[:], in_=wseg8_d[:])
        idx_sb = const.tile([128, NBLK, 8 * T], i16, tag="idx")
        nc.sync.dma_start(out=idx_sb[:], in_=idx_d[:])
        w0_sb = const.tile([128, P4, 2, H], f16, tag="w0")
        for p in range(P4):
            nc.sync.dma_start(out=w0_sb[:, p, :, :], in_=w0_d[p])
        w1_sb = const.tile([128, 8, 2, 128], f16, tag="w1")
        nc.sync.dma_start(out=w1_sb[:], in_=w1_d[:])
        w2_sb = const.tile([128, 2, 1, 128], f16, tag="w2")
        nc.sync.dma_start(out=w2_sb[:], in_=w2_d[:])
        w3_sb = const.tile([128, 1], f16, tag="w3")
        nc.sync.dma_start(out=w3_sb[:], in_=w3_d[:])
        if nontriv["b_in"]:
            b_in_sb = const.tile([128, H], f32, tag="b_in")
            nc.sync.dma_start(out=b_in_sb[:], in_=b_in_d[:])
        if nontriv["bcat"]:
            bcat_sb = const.tile([128, L, PH], f32, tag="bcat")
            for i in range(L):
                nc.sync.dma_start(out=bcat_sb[:, i, :], in_=bcat_d[i])
        if nontriv["ln"]:
            lng_sb = const.tile([128, L, PH], f32, tag="lng")
            lnb_sb = const.tile([128, L, PH], f32, tag="lnb")
            for i in range(L):
                nc.sync.dma_start(out=lng_sb[:, i, :], in_=lng_d[i])
                nc.sync.dma_start(out=lnb_sb[:, i, :], in_=lnb_d[i])
        if nontriv["b1"]:
            b1_sb = const.tile([128, 2], f32, tag="b1")
            nc.sync.dma_start(out=b1_sb[:], in_=b1_d[:])
        if nontriv["b2"]:
            b2_sb = const.tile([128, 1], f32, tag="b2")
            nc.sync.dma_start(out=b2_sb[:], in_=b2_d[:])

        # persistent activations. During layer 0, hT[:, 2p:2p+2, :] holds the
        # feature-major transpose of A^p h (the hops' projection operands);
        # after each layernorm it holds the feature-major layer output.
        hT = big.tile([128, 8, NB], f16, tag="hT")
        hcat = big.tile([128, NBLK, PH], f16, tag="hcat")
        # node-major staging slabs for the next AllGather input (batched
        # write): fp16 for 256-wide slabs, fp8 for the wide (768/512) ones
        stg = big.tile([128, NBLK, 2 * H], f16, tag="stg")
        stg8 = big.tile([128, NBLK, 2 * H], f8, tag="stg8")

        def zb(nb):
            return zero_sb[:nb, 0:1]

        def allgather(name):
            """Halo exchange ag_in[name] -> table[name]. With collectives off
            (cost-model timing builds, which can't model ncfw), stand in a
            local DMA with the same per-core HBM write volume."""
            if use_collectives:
                nc.gpsimd.collective_compute(
                    "AllGather", AluOpType.bypass, replica_groups=RG,
                    ins=[ag_in[name][:]], outs=[table[name][:]],
                )
            else:
                for c in range(2):
                    nc.sync.dma_start(
                        out=table[name][c * NB:(c + 1) * NB, :],
                        in_=ag_in[name][:])

        def stage_flush(name, width, slab):
            """Write the staged [NB, width] slab to ag_in[name] in 2 DMAs."""
            full = (NBLK - 1) * BLK  # 1152
            nc.sync.dma_start(
                out=ag_in[name][:full, :].rearrange("(a p) f -> p a f", p=128),
                in_=slab[:, : NBLK - 1, :width])
            nc.sync.dma_start(
                out=ag_in[name][full:, :],
                in_=slab[:LAST, NBLK - 1, :width])

        def transposes_to(dst_ap3, src_ap2, nb, nkt):
            """dst[128, nkt, nb] (feature-major) = per-k-tile transposes of
            src[nb, nkt*128] via PE, staged through one PSUM bank and copied
            out with a single (strided) DVE copy."""
            pst = pstr.tile([128, nkt, 128], f16, tag="tr")
            for kt in range(nkt):
                nc.tensor.transpose(pst[:, kt, :nb],
                                    src_ap2[:nb, kt * 128:(kt + 1) * 128],
                                    ident_sb[:nb, :nb])
            nc.vector.tensor_copy(dst_ap3, pst[:, :, :nb])

        def seg_psums(tabs, b):
            """Propagation block b: dma_gather the block's (deduplicated)
            source rows of each table in `tabs` into k-tiles, then
            segment-matmul on PE. Each entry of `tabs` is (tab, views)
            where views = [(byte_lo, dtype, ncols)] reinterprets byte
            ranges of the gathered rows (for the packed fp16|fp8 z1
            table); plain tables pass one full-width view. Psum pieces
            are aligned to view boundaries; returns [(c0, cw, psum_tile)]
            over the concatenated column space."""
            outs = []
            srcs = []          # (tab_idx, view) per psum piece
            base = 0
            for ti, (tab, views) in enumerate(tabs):
                for (blo, vdt, ncols) in views:
                    c0 = 0
                    while c0 < ncols:
                        cw = min(512, ncols - c0)
                        ps = psum.tile([128, 512], f32, tag="mm",
                                       name="ps_seg")
                        outs.append((base + c0, cw, ps))
                        srcs.append((ti, blo, vdt, c0))
                        c0 += cw
                    base += ncols
            for (t0, t1) in CH:
                gts = []
                for (tab, views) in tabs:
                    width = tab.shape[1]
                    gt = gpool.tile([128, t1 - t0, width], tab.dtype,
                                    tag="gath")
                    nc.gpsimd.dma_gather(
                        out_ap=gt[:],
                        in_ap=tab[:],
                        idxs_ap=idx_sb[:, b, t0 * 8: t1 * 8],
                        num_idxs=(t1 - t0) * 128,
                        num_idxs_reg=(t1 - t0) * 128,
                        elem_size=width,
                    )
                    gts.append((gt, tab.dtype))
                for (_, cw, ps), (ti, blo, vdt, tc0) in zip(outs, srcs):
                    gt, gdt = gts[ti]
                    gsz = mybir.dt.size(gdt)
                    lo = (blo + tc0 * mybir.dt.size(vdt)) // gsz
                    ncnt = cw * mybir.dt.size(vdt) // gsz
                    if vdt == f8 and (t1 - t0) % 2 == 0:
                        # fp8 pairs: DoubleRow contracts two k-tiles per
                        # instruction at 0.5 cycles/row
                        for q in range(t0, t1, 2):
                            rhs = gt[:, q - t0: q - t0 + 2,
                                     lo: lo + ncnt].bitcast(vdt)
                            nc.tensor.matmul(
                                ps[:, :cw],
                                wseg8_sb[:, b, q: q + 2, :],
                                rhs,
                                start=(q == 0),
                                stop=(q == T - 1 - 1),
                                perf_mode=mybir.MatmulPerfMode.DoubleRow,
                            )
                    else:
                        for t in range(t0, t1):
                            rhs = gt[:, t - t0, lo: lo + ncnt]
                            if vdt != gdt:
                                rhs = rhs.bitcast(vdt)
                            nc.tensor.matmul(
                                ps[:, :cw],
                                wseg_sb[:, b, t, :],
                                rhs,
                                start=(t == 0),
                                stop=(t == T - 1),
                            )
            return outs

        def ln_pass1(layer, b):
            """hcat[:, b] -> (+bias) -> bn stats -> mv (mu, var, std, rstd,
            -mu*rstd). Issued inside the last hop/projection of the layer so
            the Sqrts run while ACT is otherwise idle (one table load)."""
            hc = hcat[:, b, :]
            if nontriv["bcat"]:
                nc.vector.tensor_tensor(hc, hc, bcat_sb[:, layer, :],
                                        AluOpType.add)
            st = work.tile([128, 12], f32, tag="bnst", name="st")
            nc.vector.bn_stats(st[:, 0:6], hcat[:, b, 0:512])
            nc.vector.bn_stats(st[:, 6:12], hcat[:, b, 512:1024])
            mv = work.tile([128, 6], f32, tag=f"bnmv{b}", name="mv")
            nc.vector.bn_aggr(mv[:, 0:2], st[:])
            nc.scalar.activation(mv[:, 2:3], mv[:, 1:2], AF.Sqrt,
                                 bias=eps_sb[:, 0:1])
            nc.vector.reciprocal(mv[:, 3:4], mv[:, 2:3])
            # mv[:,4] = -mu * rstd, the fused-activation bias
            nc.vector.tensor_scalar(mv[:, 4:5], mv[:, 0:1], mv[:, 3:4],
                                    -1.0, AluOpType.mult, AluOpType.mult)
            return mv

        def ln_pass2(layer, mvs, next_blk=None):
            """normalize+gelu (one fused ACT op per block) -> hT transposes,
            interleaving `next_blk(b)` (next-layer projections / MLP chunks)
            so PE never drains between blocks."""
            for b in range(NBLK):
                nb = _nb_of(b)
                mv = mvs[b]
                gl = work.tile([128, PH], f16, tag="gel")
                if nontriv["ln"]:
                    xn = one.tile([128, PH], f32, tag="xn")
                    nc.vector.tensor_scalar(
                        xn[:], hcat[:, b, :], mv[:, 0:1], mv[:, 3:4],
                        AluOpType.subtract, AluOpType.mult,
                    )
                    nc.vector.tensor_tensor(xn[:], xn[:],
                                            lng_sb[:, layer, :],
                                            AluOpType.mult)
                    nc.vector.tensor_tensor(xn[:], xn[:],
                                            lnb_sb[:, layer, :],
                                            AluOpType.add)
                    nc.scalar.activation(gl[:], xn[:], AF.Gelu, bias=zb(128))
                else:
                    # gelu((x - mu) * rstd) == gelu(x * rstd + (-mu * rstd))
                    nc.scalar.activation(gl[:], hcat[:, b, :], AF.Gelu,
                                         bias=mv[:, 4:5], scale=mv[:, 3:4])
                transposes_to(hT[:, :, b * BLK: b * BLK + nb], gl, nb, 8)
                if next_blk is not None:
                    next_blk(b)

        # ================= stage 0: h0 = gelu(x @ w_in + b_in) =============
        for b in range(NBLK):
            nb = _nb_of(b)
            ps = psum.tile([128, 512], f32, tag="mm")
            nc.tensor.matmul(ps[:nb, :H],
                             xT_sb[:, b * BLK: b * BLK + nb],
                             w_in_sb[:F_IN, 0, :], start=True, stop=True)
            if nontriv["b_in"]:
                tmp = work.tile([128, 512], f32, tag="btmp")
                nc.vector.tensor_tensor(tmp[:nb, :H], ps[:nb, :H],
                                        b_in_sb[:nb, :], AluOpType.add)
                nc.scalar.activation(stg[:nb, b, :H], tmp[:nb, :H], AF.Gelu,
                                     bias=zb(nb))
            else:
                nc.scalar.activation(stg[:nb, b, :H], ps[:nb, :H], AF.Gelu,
                                     bias=zb(nb))
            transposes_to(hT[:, 0:2, b * BLK: b * BLK + nb],
                          stg[:, b, :H], nb, 2)
        stage_flush("l0h0", H, stg)
        allgather("l0h0")

        # ================= layer 0: propagate-then-project =================
        mvs = [None] * NBLK

        def l0_project(p):
            """hcat[:, b, p*H:(p+1)*H] = h_p @ mh_w0[p] from hT[:, 2p:2p+2].
            The p=3 pass completes hcat, so it chains each block's LN
            stats (ln_pass1) right behind its projection."""
            for b in range(NBLK):
                nb = _nb_of(b)
                ps = psum.tile([128, 512], f32, tag="mm")
                for kt in range(2):
                    nc.tensor.matmul(ps[:nb, :H],
                                     hT[:, 2 * p + kt, b * BLK: b * BLK + nb],
                                     w0_sb[:, p, kt, :],
                                     start=(kt == 0), stop=(kt == 1))
                nc.vector.tensor_copy(hcat[:nb, b, p * H:(p + 1) * H],
                                      ps[:nb, :H])

        l0_project(0)
        hops = [("l0h0", "l0h1"), ("l0h1", "l0h2"), ("l0h2", None)]
        for p, (tin, tout) in enumerate(hops, start=1):
            for b in range(NBLK):
                nb = _nb_of(b)
                (_, _, ps), = seg_psums([(table[tin], [(0, f16, H)])], b)
                if tout is not None:
                    nc.vector.tensor_copy(stg[:, b, :H], ps[:, :H])
                    transposes_to(hT[:, 2 * p: 2 * p + 2, b * BLK: b * BLK + nb],
                                  stg[:, b, :H], nb, 2)
                else:
                    sg = work.tile([128, H], f16, tag="sg")
                    nc.vector.tensor_copy(sg[:], ps[:, :H])
                    transposes_to(hT[:, 2 * p: 2 * p + 2, b * BLK: b * BLK + nb],
                                  sg[:, :H], nb, 2)
            if tout is not None:
                stage_flush(tout, H, stg)
                allgather(tout)
            l0_project(p)

        # ================= layers 1-2: project-first ======================
        def make_proj(layer, w12_sb):
            def proj(b):
                """MixHop projections for `layer`: p=0 -> hcat, p=1 -> fp16
                z1 staging, p=2,3 -> fp8 staging (AG input)."""
                nb = _nb_of(b)
                for p in range(P4):
                    ps = psum.tile([128, 512], f32, tag="mm")
                    for kt in range(8):
                        nc.tensor.matmul(ps[:nb, :H],
                                         hT[:, kt, b * BLK: b * BLK + nb],
                                         w12_sb[:, p, kt, :],
                                         start=(kt == 0), stop=(kt == 7))
                    if p == 0:
                        nc.vector.tensor_copy(hcat[:nb, b, 0:H], ps[:nb, :H])
                    elif p == 1:
                        nc.vector.tensor_copy(stg[:nb, b, :H], ps[:nb, :H])
                    else:
                        nc.vector.tensor_copy(
                            stg8[:nb, b, (p - 2) * H: (p - 1) * H],
                            ps[:nb, :H])
            return proj

        def z1_flush_ag(zn):
            # packed z1 flush: fp16 z1 bytes [0:512), fp8 [z2|z3] [512:1024)
            agz = ag_in[zn]
            full = (NBLK - 1) * BLK
            bulk = agz[:full, :].rearrange("(a p) f -> p a f", p=128)
            nc.sync.dma_start(out=bulk[:, :, 0: 2 * H].bitcast(f16),
                              in_=stg[:, : NBLK - 1, :H])
            nc.sync.dma_start(out=bulk[:, :, 2 * H: 4 * H].bitcast(f8),
                              in_=stg8[:, : NBLK - 1, : 2 * H])
            nc.sync.dma_start(out=agz[full:, 0: 2 * H].bitcast(f16),
                              in_=stg[:LAST, NBLK - 1, :H])
            nc.sync.dma_start(out=agz[full:, 2 * H: 4 * H].bitcast(f8),
                              in_=stg8[:LAST, NBLK - 1, : 2 * H])
            allgather(zn)

        def run_hops(layer):
            """The three propagation hops of a MixHop layer; the last hop
            chains per-block LN stats."""
            zname = [f"l{layer}z1", f"l{layer}z2", f"l{layer}z3"]
            hop_tabs = [[(table[zname[0]],
                          [(0, f16, H), (2 * H, f8, 2 * H)])],
                        [(table[zname[1]], [(0, f8, 2 * H)])],
                        [(table[zname[2]], [(0, f16, H)])]]
            for hop in range(3):
                width = (3 - hop) * H
                tout = zname[hop + 1] if hop < 2 else None
                for b in range(NBLK):
                    nb = _nb_of(b)
                    pieces = seg_psums(hop_tabs[hop], b)
                    # first H columns are this hop's power output
                    nc.vector.tensor_copy(
                        hcat[:nb, b, (hop + 1) * H:(hop + 2) * H],
                        pieces[0][2][:nb, :H])
                    if tout is not None:
                        oslab = stg8 if tdt[tout] == f8 else stg
                        for (c0, cw, ps) in pieces:
                            if c0 + cw <= H:
                                continue
                            lo = max(H, c0)
                            nc.vector.tensor_copy(
                                oslab[:, b, lo - H: c0 + cw - H],
                                ps[:, lo - c0: cw])
                if tout is not None:
                    stage_flush(tout, width - H,
                                stg8 if tdt[tout] == f8 else stg)
                    allgather(tout)

        # layer 1 projections interleave with layer 0's LN pass 2
        w12_sb1 = const.tile([128, P4, 8, H], f16, tag="w12")
        for p in range(P4):
            nc.sync.dma_start(out=w12_sb1[:, p, :, :], in_=w12_d[0, p])
        for b in range(NBLK):
            mvs[b] = ln_pass1(0, b)
        ln_pass2(0, mvs)
        proj1 = make_proj(1, w12_sb1)
        for b in range(NBLK):
            proj1(b)
        z1_flush_ag("l1z1")
        run_hops(1)

        w12_sb2 = const.tile([128, P4, 8, H], f16, tag="w12")
        for p in range(P4):
            nc.sync.dma_start(out=w12_sb2[:, p, :, :], in_=w12_d[1, p])
        for b in range(NBLK):
            mvs[b] = ln_pass1(1, b)
        ln_pass2(1, mvs)
        proj2 = make_proj(2, w12_sb2)
        for b in range(NBLK):
            proj2(b)
        z1_flush_ag("l2z1")
        run_hops(2)

        # ============ final MLP, interleaved with layer 2's LN ============
        m1T = big.tile([128, 2, NB], f16, tag="stg", name="m1T")
        m2T = big.tile([128, NB], f16, tag="hcat", name="m2T")
        chunks = [(c, min(512, NB - c)) for c in range(0, NB, 512)]

        def mlp_chunk(ci):
            c0, cw = chunks[ci]
            for mt in range(2):
                ps = psum.tile([128, 512], f32, tag="mm")
                for kt in range(8):
                    nc.tensor.matmul(ps[:, :cw], w1_sb[:, kt, mt, :],
                                     hT[:, kt, c0:c0 + cw],
                                     start=(kt == 0), stop=(kt == 7))
                bias = b1_sb[:, mt:mt + 1] if nontriv["b1"] else zb(128)
                nc.scalar.activation(m1T[:, mt, c0:c0 + cw], ps[:, :cw],
                                     AF.Gelu, bias=bias)
            ps = psum.tile([128, 512], f32, tag="mm")
            for kt in range(2):
                nc.tensor.matmul(ps[:, :cw], w2_sb[:, kt, 0, :],
                                 m1T[:, kt, c0:c0 + cw],
                                 start=(kt == 0), stop=(kt == 1))
            bias = b2_sb[:, 0:1] if nontriv["b2"] else zb(128)
            nc.scalar.activation(m2T[:, c0:c0 + cw], ps[:, :cw],
                                 AF.Gelu, bias=bias)

        def mlp_blk(b):
            # chunk ci spans hT columns [ci*512, ci*512+512) -> ready once
            # blocks 0..b cover them
            if b == 3:
                mlp_chunk(0)
            elif b == 7:
                mlp_chunk(1)
            elif b == 9:
                mlp_chunk(2)

        for b in range(NBLK):
            mvs[b] = ln_pass1(2, b)
        ln_pass2(2, mvs)
        for ci in range(3):
            mlp_chunk(ci)
        ysb = big.tile([1, NB], f32, tag="ysb", name="ysb")
        for (c0, cw) in chunks:
            ps = psum.tile([128, 512], f32, tag="mm")
            nc.tensor.matmul(ps[:1, :cw], w3_sb[:, :1], m2T[:, c0:c0 + cw],
                             start=True, stop=True)
            nc.vector.tensor_copy(ysb[:1, c0:c0 + cw], ps[:1, :cw])
        nc.sync.dma_start(out=y_d[:], in_=ysb[:1, :])

    nc.compile()
    return nc


# ----------------------------------------------------------------------------
# Public entry point
# ----------------------------------------------------------------------------

_CACHE = {}


def _prep_inputs(inputs):
    x = np.asarray(inputs["x"], np.float32)
    edge_index = np.asarray(inputs["edge_index"])
    wsegT, wseg8, idx16, k_pad, perm = _graph_prep(edge_index)

    b3 = np.asarray(inputs["b3"], np.float32)
    nontriv = {
        "b_in": bool(np.any(inputs["b_in"])),
        "bcat": bool(np.any(inputs["mh_b0"]) or np.any(inputs["mh_b12"])),
        "ln": not (np.allclose(np.asarray(inputs["ln_g"]), 1.0)
                   and not np.any(inputs["ln_b"])),
        "b1": bool(np.any(inputs["b1"])),
        "b2": bool(np.any(inputs["b2"])),
    }

    shared = {
        "w_in_m": _w_moving(np.asarray(inputs["w_in"], np.float32)),
        "w0_m": np.stack([_w_moving(np.asarray(inputs["mh_w0"][p], np.float32))
                          for p in range(P4)]),
        "w12_m": np.stack([
            np.stack([_w_moving(np.asarray(inputs["mh_w12"][l, p], np.float32))
                      for p in range(P4)])
            for l in range(2)]),
        "w1_st": _w_stationary(np.asarray(inputs["w1"], np.float32)),
        "w2_st": _w_stationary(np.asarray(inputs["w2"], np.float32)),
        "w3_st": np.asarray(inputs["w3"], np.float32).astype(np.float16),
        "ident": np.eye(128, dtype=np.float16),
        "eps_bc": np.full((128, 1), EPS, np.float32),
    }
    if nontriv["b_in"]:
        shared["b_in_bc"] = np.tile(np.asarray(inputs["b_in"], np.float32),
                                    (128, 1))
    if nontriv["bcat"]:
        bcat = np.zeros((L, PH), np.float32)
        bcat[0] = np.asarray(inputs["mh_b0"], np.float32).reshape(-1)
        bcat[1] = np.asarray(inputs["mh_b12"], np.float32)[0].reshape(-1)
        bcat[2] = np.asarray(inputs["mh_b12"], np.float32)[1].reshape(-1)
        shared["bcat_bc"] = np.ascontiguousarray(
            np.broadcast_to(bcat[:, None, :], (L, 128, PH)))
    if nontriv["ln"]:
        shared["lng_bc"] = np.ascontiguousarray(np.broadcast_to(
            np.asarray(inputs["ln_g"], np.float32)[:, None, :], (L, 128, PH)))
        shared["lnb_bc"] = np.ascontiguousarray(np.broadcast_to(
            np.asarray(inputs["ln_b"], np.float32)[:, None, :], (L, 128, PH)))
    if nontriv["b1"]:
        shared["b1_c"] = np.ascontiguousarray(
            np.asarray(inputs["b1"], np.float32).reshape(2, 128).T)
    if nontriv["b2"]:
        shared["b2_c"] = np.asarray(inputs["b2"], np.float32).reshape(128, 1)

    xp = x[np.argsort(perm)]  # xp[newid] = x[orig]
    in_maps = []
    for c in range(NC):
        m = dict(shared)
        m["xT"] = np.ascontiguousarray(
            xp[c * NB:(c + 1) * NB].T.astype(np.float16))
        m["idx16"] = np.ascontiguousarray(idx16[c])
        m["wsegT"] = np.ascontiguousarray(wsegT[c])
        m["wseg8"] = np.ascontiguousarray(wseg8[c])
        in_maps.append(m)
    return in_maps, k_pad, nontriv, b3, perm


def _run(inputs, trace=False, **kwargs):
    from concourse.bass_utils import run_bass_kernel_spmd

    in_maps, k_pad, nontriv, b3, perm = _prep_inputs(inputs)
    key = (k_pad, tuple(sorted(nontriv.items())))
    if key not in _CACHE:
        _CACHE[key] = _build_nc(k_pad, nontriv)
    nc = _CACHE[key]
    res = run_bass_kernel_spmd(nc, in_maps, list(range(NC)), trace=trace,
                               **kwargs)
    ycat = np.concatenate([res.results[c]["y_out"] for c in range(NC)])
    y = ycat[perm].astype(np.float32) + b3.reshape(-1)[0]
    return y, res


def kernel(**inputs) -> np.ndarray:
    y, _ = _run(inputs, trace=False)
    return y


# revision 23
# speedup vs baseline: 1.0916x; 1.0160x over previous
"""MixHopVolatilityNet Trainium2 kernel (8 NeuronCores, SPMD).

Strategy (graph/data parallel, per sharding hint):
 - Nodes partitioned across 8 cores (1250 each) via a degree-balanced
   permutation; each core owns the destination side of every propagation
   for its nodes. Weights replicated.
 - Halo exchange: after each hop every core AllGathers its 1250-row slab
   into the next full [10000, F] feature table (on-chip ncfw collective).
 - Every hop runs as gather + segment-matmul per 128-dst-node block: the
   (deduplicated, per-block) source rows of the replicated table are
   batch-gathered into SBUF k-tiles with gpsimd.dma_gather (one Q7
   instruction per <=1024 rows, ~1us desc-gen each instead of the ~17us
   of per-128-row indirect DMAs), then PE accumulates
   wsegT[k_slot, dst]^T @ gathered[k_slot, :] over the k-tiles.
 - Layer 0 propagates h directly (propagate-then-project, 3x256-wide
   hops). Layers 1-2 project first (out_p = A^p (h @ W_p) + b_p),
   batching powers into [z1|z2|z3] so hops are 768/512/256 wide.
 - Per-hop staging of the AllGather input slab is batched into one SBUF
   tile and written with 2 DMAs (not 10) to cut HWDGE/queue overhead.
 - All matmul operands fp16 (PSUM accumulates fp32); layernorm (two-pass,
   bn_stats/bn_aggr) in fp32; erf-gelu via the ACT Gelu LUT.
"""

import heapq
import sys

import numpy as np

sys.path.insert(0, "/opt/trn_rl_repo")

# ---- problem constants (hardcoded per contract) ----
N = 10000
E = 160000
F_IN = 84
H = 256
P4 = 4
L = 3
PH = P4 * H  # 1024
NC = 8
NB = N // NC          # 1250 nodes per core
BLK = 128
NBLK = (NB + BLK - 1) // BLK   # 10 blocks; the last one holds 98 nodes
LAST = NB - (NBLK - 1) * BLK   # 98
EPS = 1e-5
GMAX = 8              # k-tiles per dma_gather (1024 rows = SWDGE ring cap)


def _nb_of(b):
    return min(BLK, NB - b * BLK)


# ----------------------------------------------------------------------------
# Host-side preprocessing
# ----------------------------------------------------------------------------

def _balance_nodes(wt):
    """Greedy LPT assignment of nodes to the 80 (core, block) bins so the
    per-block gather work is balanced. Returns perm: orig node -> new id."""
    nbins = NC * NBLK
    cap = np.full(nbins, BLK, np.int64)
    cap[NBLK - 1:: NBLK] = LAST
    order = np.argsort(-wt, kind="stable")
    heap = [(0, b) for b in range(nbins)]
    heapq.heapify(heap)
    fill = np.zeros(nbins, np.int64)
    perm = np.empty(N, np.int64)
    base = np.arange(nbins) // NBLK * NB + np.arange(nbins) % NBLK * BLK
    for node in order:
        while True:
            load, b = heapq.heappop(heap)
            if fill[b] < cap[b]:
                break
        perm[node] = base[b] + fill[b]
        fill[b] += 1
        if fill[b] < cap[b]:
            heapq.heappush(heap, (load + int(wt[node]), b))
    return perm


def _graph_prep(edge_index):
    """Build per-core gather index planes and dense segment-weight blocks,
    with dst-side node balancing and per-block source deduplication."""
    src = edge_index[0].astype(np.int64)
    dst = edge_index[1].astype(np.int64)
    deg = np.bincount(dst, minlength=N).astype(np.float64) + 1.0
    dinv = deg ** -0.5
    loop = np.arange(N, dtype=np.int64)
    esrc = np.concatenate([src, loop])
    edst = np.concatenate([dst, loop])
    ew = (dinv[esrc] * dinv[edst]).astype(np.float32)

    perm = _balance_nodes(deg)  # deg ~ per-dst gather row count
    psrc = perm[esrc]
    pdst = perm[edst]

    core = pdst // NB
    loc = pdst - core * NB
    blk = loc // BLK
    m = loc - blk * BLK
    gid = core * NBLK + blk
    order = np.argsort(gid, kind="stable")
    psrc, ew, m, gid = psrc[order], ew[order], m[order], gid[order]
    starts = np.searchsorted(gid, np.arange(NC * NBLK))
    ends = np.concatenate([starts[1:], [len(gid)]])

    # per-block dedup of gather sources
    uniq_lists = []
    kk = np.empty(len(gid), np.int64)
    counts = np.empty(NC * NBLK, np.int64)
    for g in range(NC * NBLK):
        s, e = starts[g], ends[g]
        u, inv = np.unique(psrc[s:e], return_inverse=True)
        uniq_lists.append(u)
        kk[s:e] = inv
        counts[g] = len(u)

    k_pad = int(np.ceil(max(counts.max(), 128) / 128.0) * 128)
    T = k_pad // 128
    # per-block-index gather row count: max over cores, rounded up to 16
    # (the dma_gather index channels); trailing k-slots stay zero-weighted
    cnt2 = counts.reshape(NC, NBLK)
    ni_blk = tuple(int(-(-cnt2[:, b].max() // 16) * 16) for b in range(NBLK))

    wsegT = np.zeros((NC, 128, NBLK, T, BLK), np.float32)
    core_g = gid // NBLK
    blk_g = gid % NBLK
    np.add.at(wsegT, (core_g, kk % 128, blk_g, kk // 128, m), ew)
    import ml_dtypes
    wseg8 = wsegT.astype(ml_dtypes.float8_e4m3fn)
    wsegT = wsegT.astype(np.float16)

    # dma_gather index planes: slot i = t*128+p lands at out[i%128, i//128];
    # the ucode reads idxs[i] from plane[i%16, i//16], replicated across all
    # eight 16-partition groups (each Q7 core reads its own partitions).
    idx16 = np.zeros((NC, 128, NBLK, 8 * T), np.int16)
    for g in range(NC * NBLK):
        u = uniq_lists[g]
        arr = np.zeros(k_pad, np.int16)
        arr[: len(u)] = u.astype(np.int16)
        plane = arr.reshape(8 * T, 16).T          # [16, 8T]
        idx16[g // NBLK, :, g % NBLK, :] = np.tile(plane, (8, 1))
    return wsegT, wseg8, idx16, k_pad, ni_blk, perm


def _w_moving(w):
    """[K, Nout] -> moving layout [128, Kt, Nout] fp16 (partition = K % 128)."""
    K, Nout = w.shape
    Kt = (K + 127) // 128
    out = np.zeros((128, Kt, Nout), np.float16)
    for t in range(Kt):
        rows = w[t * 128: min((t + 1) * 128, K)]
        out[: rows.shape[0], t] = rows.astype(np.float16)
    return out


def _w_stationary(w):
    """[K, M] -> stationary tiles [128, Kt, Mt, 128] fp16."""
    K, M = w.shape
    Kt = (K + 127) // 128
    Mt = (M + 127) // 128
    out = np.zeros((128, Kt, Mt, 128), np.float16)
    for t in range(Kt):
        for u in range(Mt):
            blk = w[t * 128: min((t + 1) * 128, K),
                    u * 128: min((u + 1) * 128, M)].astype(np.float16)
            out[: blk.shape[0], t, u, : blk.shape[1]] = blk
    return out


# ----------------------------------------------------------------------------
# Bass program
# ----------------------------------------------------------------------------


def _build_nc(k_pad, ni_blk, nontriv, use_collectives=True):
    import concourse.bacc as bacc
    import concourse.bass as bass
    import concourse.mybir as mybir
    import concourse.tile as tile
    from concourse.alu_op_type import AluOpType
    from contextlib import ExitStack

    f16 = mybir.dt.float16
    f32 = mybir.dt.float32
    f8 = mybir.dt.float8e4
    u8 = mybir.dt.uint8
    i16 = mybir.dt.int16
    AF = mybir.ActivationFunctionType
    T = k_pad // 128
    CH = [(q * GMAX, min((q + 1) * GMAX, T))
          for q in range((T + GMAX - 1) // GMAX)]
    RG = [list(range(NC))]

    nc = bacc.Bacc("TRN2", target_bir_lowering=False, debug=False,
                   num_devices=NC)

    # ---- I/O ----
    xT_d = nc.dram_tensor("xT", [F_IN, NB], f16, kind="ExternalInput")
    idx_d = nc.dram_tensor("idx16", [128, NBLK, 8 * T], i16,
                           kind="ExternalInput")
    wseg_d = nc.dram_tensor("wsegT", [128, NBLK, T, BLK], f16,
                            kind="ExternalInput")
    wseg8_d = nc.dram_tensor("wseg8", [128, NBLK, T, BLK], f8,
                             kind="ExternalInput")
    w_in_d = nc.dram_tensor("w_in_m", [128, 1, H], f16, kind="ExternalInput")
    w0_d = nc.dram_tensor("w0_m", [P4, 128, 2, H], f16, kind="ExternalInput")
    w12_d = nc.dram_tensor("w12_m", [2, P4, 128, 8, H], f16,
                           kind="ExternalInput")
    w1_d = nc.dram_tensor("w1_st", [128, 8, 2, 128], f16, kind="ExternalInput")
    w2_d = nc.dram_tensor("w2_st", [128, 2, 1, 128], f16, kind="ExternalInput")
    w3_d = nc.dram_tensor("w3_st", [128, 1], f16, kind="ExternalInput")
    ident_d = nc.dram_tensor("ident", [128, 128], f16, kind="ExternalInput")
    eps_d = nc.dram_tensor("eps_bc", [128, 1], f32, kind="ExternalInput")
    if nontriv["b_in"]:
        b_in_d = nc.dram_tensor("b_in_bc", [128, H], f32, kind="ExternalInput")
    if nontriv["bcat"]:
        bcat_d = nc.dram_tensor("bcat_bc", [L, 128, PH], f32,
                                kind="ExternalInput")
    if nontriv["ln"]:
        lng_d = nc.dram_tensor("lng_bc", [L, 128, PH], f32,
                               kind="ExternalInput")
        lnb_d = nc.dram_tensor("lnb_bc", [L, 128, PH], f32,
                               kind="ExternalInput")
    if nontriv["b1"]:
        b1_d = nc.dram_tensor("b1_c", [128, 2], f32, kind="ExternalInput")
    if nontriv["b2"]:
        b2_d = nc.dram_tensor("b2_c", [128, 1], f32, kind="ExternalInput")
    y_d = nc.dram_tensor("y_out", [NB], f32, kind="ExternalOutput")

    # ---- internal DRAM: AG inputs (local) and gather tables (shared) ----
    # fp8 plan: the z1 stage rows are PACKED bytes [256 fp16 | 512 fp8]:
    # the z1 columns (which feed power 1 directly and are most
    # error-sensitive) stay fp16, while [z2|z3] travel fp8e4m3 (their fp8
    # noise is carried through 2-3 further propagation hops); one u8
    # gather per chunk serves both via bitcast views. The [Az2|Az3] stage
    # is fp8 too. 256-wide tables stay fp16 (sub-512B descriptors pay a
    # 2x latency multiplier, so fp8 would not shrink their time).
    ag_in = {}
    table = {}
    tdt = {}
    for name, width, dt in [
            ("l0h0", H, f16), ("l0h1", H, f16), ("l0h2", H, f16),
            ("l1z1", 4 * H, u8), ("l1z2", 2 * H, f8), ("l1z3", H, f16),
            ("l2z1", 4 * H, u8), ("l2z2", 2 * H, f8), ("l2z3", H, f16)]:
        tdt[name] = dt
        ag_in[name] = nc.dram_tensor(f"agin_{name}", [NB, width], dt)
        table[name] = nc.dram_tensor(f"tab_{name}", [N, width], dt,
                                     addr_space="Shared")

    with tile.TileContext(nc) as tc, ExitStack() as ctx:
        const = ctx.enter_context(tc.tile_pool(name="const", bufs=1))
        work = ctx.enter_context(tc.tile_pool(name="work", bufs=2))
        big = ctx.enter_context(tc.tile_pool(name="big", bufs=1))
        gpool = ctx.enter_context(tc.tile_pool(name="gpool", bufs=3))
        one = ctx.enter_context(tc.tile_pool(name="one", bufs=1))
        psum = ctx.enter_context(tc.tile_pool(name="psum", bufs=6,
                                              space="PSUM"))
        pstr = ctx.enter_context(tc.tile_pool(name="pstr", bufs=2,
                                              space="PSUM"))

        # ---- persistent SBUF constants (h0 operands first) ----
        xT_sb = const.tile([F_IN, NB], f16, tag="xT")
        nc.sync.dma_start(out=xT_sb[:], in_=xT_d[:])
        w_in_sb = const.tile([128, 1, H], f16, tag="w_in")
        nc.sync.dma_start(out=w_in_sb[:], in_=w_in_d[:])
        ident_sb = const.tile([128, 128], f16, tag="ident")
        nc.sync.dma_start(out=ident_sb[:], in_=ident_d[:])
        eps_sb = const.tile([128, 1], f32, tag="eps")
        nc.sync.dma_start(out=eps_sb[:], in_=eps_d[:])
        zero_sb = const.tile([128, 1], f32, tag="zero")
        nc.vector.memset(zero_sb[:], 0.0)
        wseg_sb = const.tile([128, NBLK, T, BLK], f16, tag="wseg")
        nc.sync.dma_start(out=wseg_sb[:], in_=wseg_d[:])
        wseg8_sb = const.tile([128, NBLK, T, BLK], f8, tag="wseg8")
        nc.sync.dma_start(out=wseg8_sb[:], in_=wseg8_d[:])
        idx_sb = const.tile([128, NBLK, 8 * T], i16, tag="idx")
        nc.sync.dma_start(out=idx_sb[:], in_=idx_d[:])
        w0_sb = const.tile([128, P4, 2, H], f16, tag="w0")
        for p in range(P4):
            nc.sync.dma_start(out=w0_sb[:, p, :, :], in_=w0_d[p])
        w1_sb = const.tile([128, 8, 2, 128], f16, tag="w1")
        nc.sync.dma_start(out=w1_sb[:], in_=w1_d[:])
        w2_sb = const.tile([128, 2, 1, 128], f16, tag="w2")
        nc.sync.dma_start(out=w2_sb[:], in_=w2_d[:])
        w3_sb = const.tile([128, 1], f16, tag="w3")
        nc.sync.dma_start(out=w3_sb[:], in_=w3_d[:])
        if nontriv["b_in"]:
            b_in_sb = const.tile([128, H], f32, tag="b_in")
            nc.sync.dma_start(out=b_in_sb[:], in_=b_in_d[:])
        if nontriv["bcat"]:
            bcat_sb = const.tile([128, L, PH], f32, tag="bcat")
            for i in range(L):
                nc.sync.dma_start(out=bcat_sb[:, i, :], in_=bcat_d[i])
        if nontriv["ln"]:
            lng_sb = const.tile([128, L, PH], f32, tag="lng")
            lnb_sb = const.tile([128, L, PH], f32, tag="lnb")
            for i in range(L):
                nc.sync.dma_start(out=lng_sb[:, i, :], in_=lng_d[i])
                nc.sync.dma_start(out=lnb_sb[:, i, :], in_=lnb_d[i])
        if nontriv["b1"]:
            b1_sb = const.tile([128, 2], f32, tag="b1")
            nc.sync.dma_start(out=b1_sb[:], in_=b1_d[:])
        if nontriv["b2"]:
            b2_sb = const.tile([128, 1], f32, tag="b2")
            nc.sync.dma_start(out=b2_sb[:], in_=b2_d[:])

        # pre-zero the rotating gather buffers: k-slots past a block's
        # gather count are never written and must stay finite (their wseg
        # weights are zero)
        for _ in range(3):
            gz = gpool.tile([128, GMAX, 4 * H], u8, tag="gath", name="gz")
            nc.vector.memset(gz[:], 0)

        # persistent activations. During layer 0, hT[:, 2p:2p+2, :] holds the
        # feature-major transpose of A^p h (the hops' projection operands);
        # after each layernorm it holds the feature-major layer output.
        hT = big.tile([128, 8, NB], f16, tag="hT")
        hcat = big.tile([128, NBLK, PH], f16, tag="hcat")
        # node-major staging slabs for the next AllGather input (batched
        # write): fp16 for 256-wide slabs, fp8 for the wide (768/512) ones
        stg = big.tile([128, NBLK, 2 * H], f16, tag="stg")
        stg8 = big.tile([128, NBLK, 2 * H], f8, tag="stg8")

        def zb(nb):
            return zero_sb[:nb, 0:1]

        def allgather(name):
            """Halo exchange ag_in[name] -> table[name]. With collectives off
            (cost-model timing builds, which can't model ncfw), stand in a
            local DMA with the same per-core HBM write volume."""
            if use_collectives:
                nc.gpsimd.collective_compute(
                    "AllGather", AluOpType.bypass, replica_groups=RG,
                    ins=[ag_in[name][:]], outs=[table[name][:]],
                )
            else:
                for c in range(2):
                    nc.sync.dma_start(
                        out=table[name][c * NB:(c + 1) * NB, :],
                        in_=ag_in[name][:])

        def stage_flush(name, width, slab):
            """Write the staged [NB, width] slab to ag_in[name] in 2 DMAs."""
            full = (NBLK - 1) * BLK  # 1152
            nc.sync.dma_start(
                out=ag_in[name][:full, :].rearrange("(a p) f -> p a f", p=128),
                in_=slab[:, : NBLK - 1, :width])
            nc.sync.dma_start(
                out=ag_in[name][full:, :],
                in_=slab[:LAST, NBLK - 1, :width])

        def transposes_to(dst_ap3, src_ap2, nb, nkt):
            """dst[128, nkt, nb] (feature-major) = per-k-tile transposes of
            src[nb, nkt*128] via PE, staged through one PSUM bank and copied
            out with a single (strided) DVE copy."""
            pst = pstr.tile([128, nkt, 128], f16, tag="tr")
            for kt in range(nkt):
                nc.tensor.transpose(pst[:, kt, :nb],
                                    src_ap2[:nb, kt * 128:(kt + 1) * 128],
                                    ident_sb[:nb, :nb])
            nc.vector.tensor_copy(dst_ap3, pst[:, :, :nb])

        def seg_psums(tabs, b):
            """Propagation block b: dma_gather the block's (deduplicated)
            source rows of each table in `tabs` into k-tiles, then
            segment-matmul on PE. Each entry of `tabs` is (tab, views)
            where views = [(byte_lo, dtype, ncols)] reinterprets byte
            ranges of the gathered rows (for the packed fp16|fp8 z1
            table); plain tables pass one full-width view. Psum pieces
            are aligned to view boundaries; returns [(c0, cw, psum_tile)]
            over the concatenated column space."""
            outs = []
            srcs = []          # (tab_idx, view) per psum piece
            base = 0
            for ti, (tab, views) in enumerate(tabs):
                for (blo, vdt, ncols) in views:
                    c0 = 0
                    while c0 < ncols:
                        cw = min(512, ncols - c0)
                        ps = psum.tile([128, 512], f32, tag="mm",
                                       name="ps_seg")
                        outs.append((base + c0, cw, ps))
                        srcs.append((ti, blo, vdt, c0))
                        c0 += cw
                    base += ncols
            for (t0, t1) in CH:
                nidx = min(ni_blk[b] - t0 * 128, (t1 - t0) * 128)
                ngr = (nidx + 127) // 128
                gts = []
                for (tab, views) in tabs:
                    width = tab.shape[1]
                    gt = gpool.tile([128, t1 - t0, width], tab.dtype,
                                    tag="gath")
                    nc.gpsimd.dma_gather(
                        out_ap=gt[:, :ngr, :],
                        in_ap=tab[:],
                        idxs_ap=idx_sb[:, b, t0 * 8: t0 * 8 + (nidx + 15) // 16],
                        num_idxs=nidx,
                        num_idxs_reg=nidx,
                        elem_size=width,
                    )
                    gts.append((gt, tab.dtype))
                for (_, cw, ps), (ti, blo, vdt, tc0) in zip(outs, srcs):
                    gt, gdt = gts[ti]
                    gsz = mybir.dt.size(gdt)
                    lo = (blo + tc0 * mybir.dt.size(vdt)) // gsz
                    ncnt = cw * mybir.dt.size(vdt) // gsz
                    if vdt == f8 and (t1 - t0) % 2 == 0:
                        # fp8 pairs: DoubleRow contracts two k-tiles per
                        # instruction at 0.5 cycles/row
                        for q in range(t0, t1, 2):
                            rhs = gt[:, q - t0: q - t0 + 2,
                                     lo: lo + ncnt].bitcast(vdt)
                            nc.tensor.matmul(
                                ps[:, :cw],
                                wseg8_sb[:, b, q: q + 2, :],
                                rhs,
                                start=(q == 0),
                                stop=(q == T - 1 - 1),
                                perf_mode=mybir.MatmulPerfMode.DoubleRow,
                            )
                    else:
                        for t in range(t0, t1):
                            rhs = gt[:, t - t0, lo: lo + ncnt]
                            if vdt != gdt:
                                rhs = rhs.bitcast(vdt)
                            nc.tensor.matmul(
                                ps[:, :cw],
                                wseg_sb[:, b, t, :],
                                rhs,
                                start=(t == 0),
                                stop=(t == T - 1),
                            )
            return outs

        def ln_pass1(layer, b):
            """hcat[:, b] -> (+bias) -> bn stats -> mv (mu, var, std, rstd,
            -mu*rstd). Issued inside the last hop/projection of the layer so
            the Sqrts run while ACT is otherwise idle (one table load)."""
            hc = hcat[:, b, :]
            if nontriv["bcat"]:
                nc.vector.tensor_tensor(hc, hc, bcat_sb[:, layer, :],
                                        AluOpType.add)
            st = work.tile([128, 12], f32, tag="bnst", name="st")
            nc.vector.bn_stats(st[:, 0:6], hcat[:, b, 0:512])
            nc.vector.bn_stats(st[:, 6:12], hcat[:, b, 512:1024])
            mv = work.tile([128, 6], f32, tag=f"bnmv{b}", name="mv")
            nc.vector.bn_aggr(mv[:, 0:2], st[:])
            nc.scalar.activation(mv[:, 2:3], mv[:, 1:2], AF.Sqrt,
                                 bias=eps_sb[:, 0:1])
            nc.vector.reciprocal(mv[:, 3:4], mv[:, 2:3])
            # mv[:,4] = -mu * rstd, the fused-activation bias
            nc.vector.tensor_scalar(mv[:, 4:5], mv[:, 0:1], mv[:, 3:4],
                                    -1.0, AluOpType.mult, AluOpType.mult)
            return mv

        def ln_pass2(layer, mvs, next_blk=None):
            """normalize+gelu (one fused ACT op per block) -> hT transposes,
            interleaving `next_blk(b)` (next-layer projections / MLP chunks)
            so PE never drains between blocks."""
            for b in range(NBLK):
                nb = _nb_of(b)
                mv = mvs[b]
                gl = work.tile([128, PH], f16, tag="gel")
                if nontriv["ln"]:
                    xn = one.tile([128, PH], f32, tag="xn")
                    nc.vector.tensor_scalar(
                        xn[:], hcat[:, b, :], mv[:, 0:1], mv[:, 3:4],
                        AluOpType.subtract, AluOpType.mult,
                    )
                    nc.vector.tensor_tensor(xn[:], xn[:],
                                            lng_sb[:, layer, :],
                                            AluOpType.mult)
                    nc.vector.tensor_tensor(xn[:], xn[:],
                                            lnb_sb[:, layer, :],
                                            AluOpType.add)
                    nc.scalar.activation(gl[:], xn[:], AF.Gelu, bias=zb(128))
                else:
                    # gelu((x - mu) * rstd) == gelu(x * rstd + (-mu * rstd))
                    nc.scalar.activation(gl[:], hcat[:, b, :], AF.Gelu,
                                         bias=mv[:, 4:5], scale=mv[:, 3:4])
                transposes_to(hT[:, :, b * BLK: b * BLK + nb], gl, nb, 8)
                if next_blk is not None:
                    next_blk(b)

        # ================= stage 0: h0 = gelu(x @ w_in + b_in) =============
        for b in range(NBLK):
            nb = _nb_of(b)
            ps = psum.tile([128, 512], f32, tag="mm")
            nc.tensor.matmul(ps[:nb, :H],
                             xT_sb[:, b * BLK: b * BLK + nb],
                             w_in_sb[:F_IN, 0, :], start=True, stop=True)
            if nontriv["b_in"]:
                tmp = work.tile([128, 512], f32, tag="btmp")
                nc.vector.tensor_tensor(tmp[:nb, :H], ps[:nb, :H],
                                        b_in_sb[:nb, :], AluOpType.add)
                nc.scalar.activation(stg[:nb, b, :H], tmp[:nb, :H], AF.Gelu,
                                     bias=zb(nb))
            else:
                nc.scalar.activation(stg[:nb, b, :H], ps[:nb, :H], AF.Gelu,
                                     bias=zb(nb))
            transposes_to(hT[:, 0:2, b * BLK: b * BLK + nb],
                          stg[:, b, :H], nb, 2)
        stage_flush("l0h0", H, stg)
        allgather("l0h0")

        # ================= layer 0: propagate-then-project =================
        mvs = [None] * NBLK

        def l0_project(p):
            """hcat[:, b, p*H:(p+1)*H] = h_p @ mh_w0[p] from hT[:, 2p:2p+2].
            The p=3 pass completes hcat, so it chains each block's LN
            stats (ln_pass1) right behind its projection."""
            for b in range(NBLK):
                nb = _nb_of(b)
                ps = psum.tile([128, 512], f32, tag="mm")
                for kt in range(2):
                    nc.tensor.matmul(ps[:nb, :H],
                                     hT[:, 2 * p + kt, b * BLK: b * BLK + nb],
                                     w0_sb[:, p, kt, :],
                                     start=(kt == 0), stop=(kt == 1))
                nc.vector.tensor_copy(hcat[:nb, b, p * H:(p + 1) * H],
                                      ps[:nb, :H])

        l0_project(0)
        hops = [("l0h0", "l0h1"), ("l0h1", "l0h2"), ("l0h2", None)]
        for p, (tin, tout) in enumerate(hops, start=1):
            for b in range(NBLK):
                nb = _nb_of(b)
                (_, _, ps), = seg_psums([(table[tin], [(0, f16, H)])], b)
                if tout is not None:
                    nc.vector.tensor_copy(stg[:, b, :H], ps[:, :H])
                    transposes_to(hT[:, 2 * p: 2 * p + 2, b * BLK: b * BLK + nb],
                                  stg[:, b, :H], nb, 2)
                else:
                    sg = work.tile([128, H], f16, tag="sg")
                    nc.vector.tensor_copy(sg[:], ps[:, :H])
                    transposes_to(hT[:, 2 * p: 2 * p + 2, b * BLK: b * BLK + nb],
                                  sg[:, :H], nb, 2)
            if tout is not None:
                stage_flush(tout, H, stg)
                allgather(tout)
            l0_project(p)

        # ================= layers 1-2: project-first ======================
        def make_proj(layer, w12_sb):
            def proj(b):
                """MixHop projections for `layer`: p=0 -> hcat, p=1 -> fp16
                z1 staging, p=2,3 -> fp8 staging (AG input)."""
                nb = _nb_of(b)
                for p in range(P4):
                    ps = psum.tile([128, 512], f32, tag="mm")
                    for kt in range(8):
                        nc.tensor.matmul(ps[:nb, :H],
                                         hT[:, kt, b * BLK: b * BLK + nb],
                                         w12_sb[:, p, kt, :],
                                         start=(kt == 0), stop=(kt == 7))
                    if p == 0:
                        nc.vector.tensor_copy(hcat[:nb, b, 0:H], ps[:nb, :H])
                    elif p == 1:
                        nc.vector.tensor_copy(stg[:nb, b, :H], ps[:nb, :H])
                    else:
                        nc.vector.tensor_copy(
                            stg8[:nb, b, (p - 2) * H: (p - 1) * H],
                            ps[:nb, :H])
            return proj

        def z1_flush_ag(zn):
            # packed z1 flush: fp16 z1 bytes [0:512), fp8 [z2|z3] [512:1024)
            agz = ag_in[zn]
            full = (NBLK - 1) * BLK
            bulk = agz[:full, :].rearrange("(a p) f -> p a f", p=128)
            nc.sync.dma_start(out=bulk[:, :, 0: 2 * H].bitcast(f16),
                              in_=stg[:, : NBLK - 1, :H])
            nc.sync.dma_start(out=bulk[:, :, 2 * H: 4 * H].bitcast(f8),
                              in_=stg8[:, : NBLK - 1, : 2 * H])
            nc.sync.dma_start(out=agz[full:, 0: 2 * H].bitcast(f16),
                              in_=stg[:LAST, NBLK - 1, :H])
            nc.sync.dma_start(out=agz[full:, 2 * H: 4 * H].bitcast(f8),
                              in_=stg8[:LAST, NBLK - 1, : 2 * H])
            allgather(zn)

        def run_hops(layer):
            """The three propagation hops of a MixHop layer; the last hop
            chains per-block LN stats."""
            zname = [f"l{layer}z1", f"l{layer}z2", f"l{layer}z3"]
            hop_tabs = [[(table[zname[0]],
                          [(0, f16, H), (2 * H, f8, 2 * H)])],
                        [(table[zname[1]], [(0, f8, 2 * H)])],
                        [(table[zname[2]], [(0, f16, H)])]]
            for hop in range(3):
                width = (3 - hop) * H
                tout = zname[hop + 1] if hop < 2 else None
                for b in range(NBLK):
                    nb = _nb_of(b)
                    pieces = seg_psums(hop_tabs[hop], b)
                    # first H columns are this hop's power output
                    nc.vector.tensor_copy(
                        hcat[:nb, b, (hop + 1) * H:(hop + 2) * H],
                        pieces[0][2][:nb, :H])
                    if tout is not None:
                        oslab = stg8 if tdt[tout] == f8 else stg
                        for (c0, cw, ps) in pieces:
                            if c0 + cw <= H:
                                continue
                            lo = max(H, c0)
                            nc.vector.tensor_copy(
                                oslab[:, b, lo - H: c0 + cw - H],
                                ps[:, lo - c0: cw])
                if tout is not None:
                    stage_flush(tout, width - H,
                                stg8 if tdt[tout] == f8 else stg)
                    allgather(tout)

        # layer 1 projections interleave with layer 0's LN pass 2
        w12_sb1 = const.tile([128, P4, 8, H], f16, tag="w12")
        for p in range(P4):
            nc.sync.dma_start(out=w12_sb1[:, p, :, :], in_=w12_d[0, p])
        for b in range(NBLK):
            mvs[b] = ln_pass1(0, b)
        ln_pass2(0, mvs)
        proj1 = make_proj(1, w12_sb1)
        for b in range(NBLK):
            proj1(b)
        z1_flush_ag("l1z1")
        run_hops(1)

        w12_sb2 = const.tile([128, P4, 8, H], f16, tag="w12")
        for p in range(P4):
            nc.sync.dma_start(out=w12_sb2[:, p, :, :], in_=w12_d[1, p])
        for b in range(NBLK):
            mvs[b] = ln_pass1(1, b)
        ln_pass2(1, mvs)
        proj2 = make_proj(2, w12_sb2)
        for b in range(NBLK):
            proj2(b)
        z1_flush_ag("l2z1")
        run_hops(2)

        # ============ final MLP, interleaved with layer 2's LN ============
        m1T = big.tile([128, 2, NB], f16, tag="stg", name="m1T")
        m2T = big.tile([128, NB], f16, tag="hcat", name="m2T")
        chunks = [(c, min(512, NB - c)) for c in range(0, NB, 512)]

        def mlp_chunk(ci):
            c0, cw = chunks[ci]
            for mt in range(2):
                ps = psum.tile([128, 512], f32, tag="mm")
                for kt in range(8):
                    nc.tensor.matmul(ps[:, :cw], w1_sb[:, kt, mt, :],
                                     hT[:, kt, c0:c0 + cw],
                                     start=(kt == 0), stop=(kt == 7))
                bias = b1_sb[:, mt:mt + 1] if nontriv["b1"] else zb(128)
                nc.scalar.activation(m1T[:, mt, c0:c0 + cw], ps[:, :cw],
                                     AF.Gelu, bias=bias)
            ps = psum.tile([128, 512], f32, tag="mm")
            for kt in range(2):
                nc.tensor.matmul(ps[:, :cw], w2_sb[:, kt, 0, :],
                                 m1T[:, kt, c0:c0 + cw],
                                 start=(kt == 0), stop=(kt == 1))
            bias = b2_sb[:, 0:1] if nontriv["b2"] else zb(128)
            nc.scalar.activation(m2T[:, c0:c0 + cw], ps[:, :cw],
                                 AF.Gelu, bias=bias)

        def mlp_blk(b):
            # chunk ci spans hT columns [ci*512, ci*512+512) -> ready once
            # blocks 0..b cover them
            if b == 3:
                mlp_chunk(0)
            elif b == 7:
                mlp_chunk(1)
            elif b == 9:
                mlp_chunk(2)

        for b in range(NBLK):
            mvs[b] = ln_pass1(2, b)
        ln_pass2(2, mvs)
        for ci in range(3):
            mlp_chunk(ci)
        ysb = big.tile([1, NB], f32, tag="ysb", name="ysb")
        for (c0, cw) in chunks:
            ps = psum.tile([128, 512], f32, tag="mm")
            nc.tensor.matmul(ps[:1, :cw], w3_sb[:, :1], m2T[:, c0:c0 + cw],
                             start=True, stop=True)
            nc.vector.tensor_copy(ysb[:1, c0:c0 + cw], ps[:1, :cw])
        nc.sync.dma_start(out=y_d[:], in_=ysb[:1, :])

    nc.compile()
    return nc


# ----------------------------------------------------------------------------
# Public entry point
# ----------------------------------------------------------------------------

_CACHE = {}


def _prep_inputs(inputs):
    x = np.asarray(inputs["x"], np.float32)
    edge_index = np.asarray(inputs["edge_index"])
    wsegT, wseg8, idx16, k_pad, ni_blk, perm = _graph_prep(edge_index)

    b3 = np.asarray(inputs["b3"], np.float32)
    nontriv = {
        "b_in": bool(np.any(inputs["b_in"])),
        "bcat": bool(np.any(inputs["mh_b0"]) or np.any(inputs["mh_b12"])),
        "ln": not (np.allclose(np.asarray(inputs["ln_g"]), 1.0)
                   and not np.any(inputs["ln_b"])),
        "b1": bool(np.any(inputs["b1"])),
        "b2": bool(np.any(inputs["b2"])),
    }

    shared = {
        "w_in_m": _w_moving(np.asarray(inputs["w_in"], np.float32)),
        "w0_m": np.stack([_w_moving(np.asarray(inputs["mh_w0"][p], np.float32))
                          for p in range(P4)]),
        "w12_m": np.stack([
            np.stack([_w_moving(np.asarray(inputs["mh_w12"][l, p], np.float32))
                      for p in range(P4)])
            for l in range(2)]),
        "w1_st": _w_stationary(np.asarray(inputs["w1"], np.float32)),
        "w2_st": _w_stationary(np.asarray(inputs["w2"], np.float32)),
        "w3_st": np.asarray(inputs["w3"], np.float32).astype(np.float16),
        "ident": np.eye(128, dtype=np.float16),
        "eps_bc": np.full((128, 1), EPS, np.float32),
    }
    if nontriv["b_in"]:
        shared["b_in_bc"] = np.tile(np.asarray(inputs["b_in"], np.float32),
                                    (128, 1))
    if nontriv["bcat"]:
        bcat = np.zeros((L, PH), np.float32)
        bcat[0] = np.asarray(inputs["mh_b0"], np.float32).reshape(-1)
        bcat[1] = np.asarray(inputs["mh_b12"], np.float32)[0].reshape(-1)
        bcat[2] = np.asarray(inputs["mh_b12"], np.float32)[1].reshape(-1)
        shared["bcat_bc"] = np.ascontiguousarray(
            np.broadcast_to(bcat[:, None, :], (L, 128, PH)))
    if nontriv["ln"]:
        shared["lng_bc"] = np.ascontiguousarray(np.broadcast_to(
            np.asarray(inputs["ln_g"], np.float32)[:, None, :], (L, 128, PH)))
        shared["lnb_bc"] = np.ascontiguousarray(np.broadcast_to(
            np.asarray(inputs["ln_b"], np.float32)[:, None, :], (L, 128, PH)))
    if nontriv["b1"]:
        shared["b1_c"] = np.ascontiguousarray(
            np.asarray(inputs["b1"], np.float32).reshape(2, 128).T)
    if nontriv["b2"]:
        shared["b2_c"] = np.asarray(inputs["b2"], np.float32).reshape(128, 1)

    xp = x[np.argsort(perm)]  # xp[newid] = x[orig]
    in_maps = []
    for c in range(NC):
        m = dict(shared)
        m["xT"] = np.ascontiguousarray(
            xp[c * NB:(c + 1) * NB].T.astype(np.float16))
        m["idx16"] = np.ascontiguousarray(idx16[c])
        m["wsegT"] = np.ascontiguousarray(wsegT[c])
        m["wseg8"] = np.ascontiguousarray(wseg8[c])
        in_maps.append(m)
    return in_maps, k_pad, ni_blk, nontriv, b3, perm


def _run(inputs, trace=False, **kwargs):
    from concourse.bass_utils import run_bass_kernel_spmd

    in_maps, k_pad, ni_blk, nontriv, b3, perm = _prep_inputs(inputs)
    key = (k_pad, ni_blk, tuple(sorted(nontriv.items())))
    if key not in _CACHE:
        _CACHE[key] = _build_nc(k_pad, ni_blk, nontriv)
    nc = _CACHE[key]
    res = run_bass_kernel_spmd(nc, in_maps, list(range(NC)), trace=trace,
                               **kwargs)
    ycat = np.concatenate([res.results[c]["y_out"] for c in range(NC)])
    y = ycat[perm].astype(np.float32) + b3.reshape(-1)[0]
    return y, res


def kernel(**inputs) -> np.ndarray:
    y, _ = _run(inputs, trace=False)
    return y
